# revision 43
# baseline (speedup 1.0000x reference)
"""Trainium2 Bass kernel for nn_MoEBlock_78288663872291 (moe_routing).

Sharding across 8 NeuronCores (single SPMD NEFF, per-core differences are
input *data* only):
  - Attention: core c handles batch c//2, query-half c%2. Host rolls each
    batch's 512-token [A|C|B] sequence so this core's 256 queries are always
    rows 0:256 (keeps the program uniform). KV projection is computed for
    the full 512 tokens (duplicated across the 2 cores of a batch). All
    attention matmuls run as float32r (full fp32 data, 1 cycle/row on the
    PE for moving dims >= 256, vs 4 for plain fp32) - the MoE top-k routing
    is numerically sensitive, and fp32r keeps it bit-stable enough.
    The directed [A|C|B] mask is a per-(key-chunk, query-chunk) 8-entry
    table fused into the softmax Exp as an activation bias. LN params /
    biases arrive as single rows and are broadcast on-chip.
  - ONE fused AllGather: each core contributes 256 bf16 LN2'd token rows
    plus 8 rows carrying the (D,8) f32 router logits bitcast into the bf16
    buffer. A single collective ~halves the per-op ncfw latency floor paid
    vs two separate AGs at LNC1 x8 ranks.
  - MoE expert-parallel: every core redundantly computes routing for all
    16 tiles in one batched pass ([P,16,8] top-1/top-2 masks, sigmoid
    gates, prefix-position counts via two 128-wide matmuls), then runs its
    expert F-slices on capacity-compacted tokens (compaction via one-hot
    G matmuls):
      unit A: expert c//2, F columns (c%2)*1024..+1024 of W1/W2
      unit B: same
      units C x3: quarter-F slices; global quarter q = 3c+u -> expert q//4,
                  F columns (q%4)*512..+512
    Expert weights (bf16) are preloaded on the sync DMA ring, gated behind
    LN1 completion so their transfers never contend with the attention-
    critical input loads.
  - Host combine: out = t_full + sum over (core, unit) of
      gate[token, expert] * y_unit[j] (y tables in bf16) scattered by the
      device-produced gate tables.
"""

import functools
import numpy as np
import ml_dtypes

import concourse.bass as bass
import concourse.bacc as bacc
import concourse.tile as tile
import concourse.mybir as mybir
from concourse.masks import make_identity

F32 = mybir.dt.float32
F32R = mybir.dt.float32r
BF16 = mybir.dt.bfloat16
FP8 = mybir.dt.float8e4
I32 = mybir.dt.int32
AF = mybir.ActivationFunctionType
ALU = mybir.AluOpType

NCORES = 8
P = 128
D = 512
NTOK = 512
NQ = 256
H = 8
HD = 64
BATCH = 4
NGLOBAL = BATCH * NTOK  # 2048

EA, EB, EC = 4, 4, 6
CAP_A, CAP_B, CAP_C = 384, 384, 256
FA, FC = 1024, 512
SENTINEL = float(1 << 20)
NEG_MASK = -30.0
EPS = 1e-5

# groups within a batch's [A|C|B] row block
GROUPS = {
    "A": dict(E=EA, K=2, cap=CAP_A, off=0, width=128),
    "C": dict(E=EC, K=1, cap=CAP_C, off=128, width=256),
    "B": dict(E=EB, K=2, cap=CAP_B, off=384, width=128),
}
# global 128-row tile i of t_full: batch i//4, chunk i%4 -> group
CHUNK_GROUP = ["A", "C", "C", "B"]
GROUP_TILES = {
    "A": [4 * b + 0 for b in range(BATCH)],
    "C": [4 * b + c for b in range(BATCH) for c in (1, 2)],
    "B": [4 * b + 3 for b in range(BATCH)],
}


# --------------------------------------------------------------------------
# device kernel
# --------------------------------------------------------------------------

def _declare_io(nc):
    t = {}
    def I(name, shape, dt=F32):
        t[name] = nc.dram_tensor(name, shape, dt, kind="ExternalInput")
    def O(name, shape, dt=F32):
        t[name] = nc.dram_tensor(name, shape, dt, kind="ExternalOutput")
    I("x_tok", [NTOK, D])            # rolled tokens of this core's batch
    # brow: [g1c(4D) | b1c(4D) | g2c(2D) | b2c(2D) | bv(D) | mrow(8)]
    I("brow", [1, 13 * D + 8])
    I("attn_w", [P, 12 * D], F32R)   # [wq | wk | wv] pre-tiled
    I("smalls", [P, 92])             # [bq(4) | bk(4) | bo(4) | wrt(64) | brt(16)]
    I("woT", [HD, H * D], F32R)
    I("w1_a", [P, 4 * FA], BF16); I("w2_a", [P, (FA // P) * D], BF16)
    I("b1_a", [P, FA // P]); I("b2_a", [P, 4])
    I("w1_b", [P, 4 * FA], BF16); I("w2_b", [P, (FA // P) * D], BF16)
    I("b1_b", [P, FA // P]); I("b2_b", [P, 4])
    I("w1_c", [3, P, 4 * FC], BF16); I("w2_c", [3, P, (FC // P) * D], BF16)
    I("b1_c", [3, P, FC // P]); I("b2_c", [3, P, 4])
    I("eoh_a", [P, EA]); I("eoh_b", [P, EB])
    I("eoh_c0", [P, EC]); I("eoh_c1", [P, EC]); I("eoh_c2", [P, EC])
    O("t_out", [NQ, D])
    O("ya_t", [P, 4 * CAP_A], BF16); O("yb_t", [P, 4 * CAP_B], BF16)
    O("yc_t", [3, P, 4 * CAP_C], BF16)
    O("gates_a", [BATCH * 128, EA])
    O("gates_b", [BATCH * 128, EB])
    O("gates_c", [BATCH * 256, EC])
    return t


def _ln_tile(nc, sm, x_sl, g_t, b_t, eps_t):
    """In-place layernorm over the free axis of x_sl (128, D)."""
    stats = sm.tile([P, nc.vector.BN_STATS_DIM], F32, tag="ln_stats")
    nc.vector.bn_stats(out=stats[:], in_=x_sl)
    mv = sm.tile([P, nc.vector.BN_AGGR_DIM], F32, tag="ln_mv")
    nc.vector.bn_aggr(out=mv[:], in_=stats[:])
    nc.scalar.activation(out=mv[:, 1:2], in_=mv[:, 1:2], func=AF.Sqrt,
                         bias=eps_t[:], scale=1.0)
    nc.vector.reciprocal(out=mv[:, 1:2], in_=mv[:, 1:2])
    nc.vector.tensor_scalar(out=x_sl, in0=x_sl, scalar1=mv[:, 0:1],
                            scalar2=mv[:, 1:2], op0=ALU.subtract, op1=ALU.mult)
    nc.vector.tensor_mul(out=x_sl, in0=x_sl, in1=g_t)
    nc.vector.tensor_add(out=x_sl, in0=x_sl, in1=b_t)


def _body(nc, tc, t, stage=4, rep=0, simulate=False):
    import contextlib
    R = f"r{rep}_"
    with contextlib.ExitStack() as top:
        const = top.enter_context(tc.tile_pool(name=R + "const", bufs=1))
        wt = top.enter_context(tc.tile_pool(name=R + "wt", bufs=1))
        sm = top.enter_context(tc.tile_pool(name=R + "sm", bufs=8))
        dram = top.enter_context(tc.tile_pool(name=R + "dram", bufs=1, space="DRAM"))
        ps_mm = top.enter_context(tc.tile_pool(name=R + "ps_mm", bufs=2, space="PSUM"))
        ps_t = top.enter_context(tc.tile_pool(name=R + "ps_t", bufs=2, space="PSUM"))
        ps_o = top.enter_context(tc.tile_pool(name=R + "ps_o", bufs=1, space="PSUM"))
        ps_x = top.enter_context(tc.tile_pool(name=R + "ps_x", bufs=1, space="PSUM"))

        ident = const.tile([P, P], F32, tag="ident")
        make_identity(nc, ident[:])
        eps_t = const.tile([P, 1], F32, tag="eps")
        nc.vector.memset(eps_t[:], EPS)
        ones128 = const.tile([P, P], F32, tag="ones128")
        nc.vector.memset(ones128[:], 1.0)
        # strict-lower-triangular (LT[p, f] = 1.0 iff p < f) for prefix counts
        LT = const.tile([P, P], F32, tag="LT")
        nc.gpsimd.memset(LT[:], 1.0)
        nc.gpsimd.affine_select(out=LT[:], in_=LT[:], compare_op=ALU.is_gt,
                                fill=0.0, base=0, pattern=[[1, P]],
                                channel_multiplier=-1)
        # iota over capacity slots (free axis), fp32 exact ints
        iotaCap_i = const.tile([P, CAP_A], I32, tag="iotaCap_i")
        nc.gpsimd.iota(iotaCap_i[:], pattern=[[1, CAP_A]], base=0, channel_multiplier=0)
        iotaCap = const.tile([P, CAP_A], F32, tag="iotaCap")
        nc.vector.tensor_copy(out=iotaCap[:], in_=iotaCap_i[:])
        # PE warm-up: ~90 tiny back-to-back matmuls keep the PE busy through
        # the LN phase so the HAM clock-gate opens (1.2 -> 2.4 GHz) before
        # the QKV matmuls. Result is kept live via the preload-gate write.
        identb = const.tile([P, P], BF16, tag="identb")
        nc.vector.tensor_copy(out=identb[:], in_=ones128[:])
        wub = const.tile([P, 1], F32, tag="wub")
        wps = ps_x.tile([P, 64], F32, tag="warm")
        for wi in range(90):
            nc.tensor.matmul(wps[:], lhsT=identb[:], rhs=identb[:, 0:64],
                             start=True, stop=True)
        nc.vector.tensor_copy(out=wub[:], in_=wps[:, 0:1])

        def bcast(src_row, tag, w=D):
            tl = const.tile([P, w], F32, tag=tag, name=tag)
            nc.gpsimd.dma_start(out=tl[:], in_=src_row.to_broadcast((P, w)))
            return tl

        # dram scratch (shared across reps so repetitions serialize)
        # ag_in rows 0:256 = bf16 tokens; rows 256:264 = f32 logits (bitcast)
        NR = NQ + 8
        if "ag_in" not in t:
            t["ag_in"] = nc.dram_tensor("ag_in", [NR, D], BF16, kind="Internal")
            t["ag_full"] = nc.dram_tensor("ag_full", [NCORES * NR, D], BF16,
                                          kind="Internal", addr_space="Shared")
        ag_in, ag_full = t["ag_in"], t["ag_full"]
        agx_in = ag_in[0:NQ, :]
        # [256, 8] f32 view of the logit rows (token-major)
        agl_in = ag_in[NQ:NR, :].bitcast(F32).rearrange("a (p e) -> (a p) e", e=8)
        ag3 = ag_full[:, :].rearrange("(c r) d -> c r d", r=NR)
        xbf_full = ag3[:, 0:NQ, :]
        log_full = ag3[:, NQ:NR, :].bitcast(F32).rearrange(
            "c a (p e) -> c (a p) e", e=8)

        if stage == 0:
            z = sm.tile([P, D], F32, tag="z0", name="z0")
            nc.sync.dma_start(out=z[:], in_=t["x_tok"][0:P, :])
            nc.sync.dma_start(out=t["t_out"][0:P, :], in_=z[:])
            return

        # ================= attention phase =================
        with contextlib.ExitStack() as aph:
            ac = aph.enter_context(tc.tile_pool(name=R + "ac", bufs=1))
            aw = aph.enter_context(tc.tile_pool(name=R + "aw", bufs=2))
            ew = aph.enter_context(tc.tile_pool(name=R + "ew", bufs=1))

            # critical-path input DMAs first, merged to minimize
            # descriptor-generation serialization on the sync engine
            x_raw = ac.tile([P, 4, D], F32, tag="x_raw")
            nc.sync.dma_start(out=x_raw[:], in_=t["x_tok"][:, :].rearrange("(c p) d -> p c d", p=P))
            attw = ac.tile([P, 3, 4, D], F32R, tag="attw")
            nc.sync.dma_start(out=attw[:], in_=t["attn_w"][:, :].rearrange("p (w o r) -> p w o r", w=3, o=4))
            woT = ac.tile([HD, H, D], F32R, tag="woT")
            nc.sync.dma_start(out=woT[:], in_=t["woT"][:, :].rearrange("p (h d) -> p h d", h=H))
            smt = ac.tile([P, 92], F32, tag="smt")
            nc.sync.dma_start(out=smt[:], in_=t["smalls"][:, :])
            brow_t = ac.tile([P, 13 * D + 8], F32, tag="brow_t")
            nc.gpsimd.dma_start(out=brow_t[:],
                                in_=t["brow"][0:1, :].to_broadcast((P, 13 * D + 8)))
            wq, wk, wv = attw[:][:, 0], attw[:][:, 1], attw[:][:, 2]
            bq, bk, bo = smt[:][:, 0:4], smt[:][:, 4:8], smt[:][:, 8:12]
            wrt = smt[:][:, 12:76].rearrange("p (c k e) -> p c k e", c=2, k=4)
            brt = smt[:][:, 76:92].rearrange("p (c e) -> p c e", c=2)
            bt = brow_t[:]
            ln1g, ln1b = bt[:, 0:4 * D], bt[:, 4 * D:8 * D]
            ln2g_o, ln2b_o = bt[:, 8 * D:10 * D], bt[:, 10 * D:12 * D]
            bv_bc = bt[:, 12 * D:13 * D]
            mrow_t = bt[:, 13 * D:13 * D + 8]

            # MoE expert-unit weight preloads: issued now so the (big) DMAs
            # overlap attention compute; tiles live in the top-level pool.
            units = []
            units.append(dict(g="A", cap=CAP_A, F=FA, tg="ua",
                              w1=t["w1_a"], w2=t["w2_a"], b1=t["b1_a"], b2=t["b2_a"],
                              yout=t["ya_t"], sel="a"))
            units.append(dict(g="B", cap=CAP_B, F=FA, tg="ub",
                              w1=t["w1_b"], w2=t["w2_b"], b1=t["b1_b"], b2=t["b2_b"],
                              yout=t["yb_t"], sel="b"))
            for u in range(3):
                units.append(dict(g="C", cap=CAP_C, F=FC, tg=f"uc{u}",
                                  w1=t["w1_c"][u], w2=t["w2_c"][u],
                                  b1=t["b1_c"][u], b2=t["b2_c"][u],
                                  yout=t["yc_t"][u], sel=f"c{u}"))
            def preload_unit(un, pool, gate):
                F, fch, tg = un["F"], un["F"] // P, un["tg"]
                un["w1_sb"] = pool.tile([P, 4, F], BF16, tag=f"w1_{tg}", name=f"w1_{tg}")
                un["w2_sb"] = pool.tile([P, fch, D], BF16, tag=f"w2_{tg}", name=f"w2_{tg}")
                # gate the big weight DMAs behind the LN1 stats so their
                # transfers can't steal bandwidth from the attention-phase
                # critical input loads (WAW dep: bypass-write, then DMA)
                for tl in (un["w1_sb"], un["w2_sb"]):
                    nc.vector.tensor_scalar(out=tl[:, 0, 0:1], in0=gate,
                                            scalar1=0.0, scalar2=None,
                                            op0=ALU.bypass)
                    nc.vector.tensor_scalar(out=tl[:, 1, 0:1], in0=wub[:],
                                            scalar1=0.0, scalar2=None,
                                            op0=ALU.bypass)
                nc.sync.dma_start(out=un["w1_sb"][:], in_=un["w1"][:, :].rearrange("p (o f) -> p o f", o=4))
                nc.sync.dma_start(out=un["w2_sb"][:], in_=un["w2"][:, :].rearrange("p (o d) -> p o d", o=fch))
                un["b1_sb"] = pool.tile([P, fch], F32, tag=f"b1_{tg}", name=f"b1_{tg}")
                nc.sync.dma_start(out=un["b1_sb"][:], in_=un["b1"][:, :])
                un["b2_sb"] = pool.tile([P, 4], F32, tag=f"b2_{tg}", name=f"b2_{tg}")
                nc.sync.dma_start(out=un["b2_sb"][:], in_=un["b2"][:, :])
                un["eoh_sb"] = pool.tile([P, 8], F32, tag=f"eoh_{tg}", name=f"eoh_{tg}")
                nc.sync.dma_start(out=un["eoh_sb"][:, :GROUPS[un["g"]]["E"]],
                                  in_=t[f"eoh_{un['sel']}"][:, :])

            with contextlib.ExitStack() as lnx:
                lnp = lnx.enter_context(tc.tile_pool(name=R + "lnp", bufs=1))
                xln = lnp.tile([P, 4, D], F32, tag="xln")
                mvall = sm.tile([P, 4, 2], F32, tag="mvall")
                for i in range(4):
                    stats = sm.tile([P, nc.vector.BN_STATS_DIM], F32, tag="ln_stats")
                    nc.vector.bn_stats(out=stats[:], in_=x_raw[:, i, :])
                    nc.vector.bn_aggr(out=mvall[:, i, :], in_=stats[:])
                sq4 = sm.tile([P, 4], F32, tag="sq4")
                nc.scalar.activation(out=sq4[:], in_=mvall[:, :, 1], func=AF.Sqrt,
                                     bias=eps_t[:], scale=1.0)
                rs4 = sm.tile([P, 4], F32, tag="rs4")
                nc.vector.reciprocal(out=rs4[:], in_=sq4[:])
                for i in range(4):
                    nc.vector.tensor_scalar(out=xln[:, i, :], in0=x_raw[:, i, :],
                                            scalar1=mvall[:, i, 0:1],
                                            scalar2=rs4[:, i:i + 1],
                                            op0=ALU.subtract, op1=ALU.mult)
                    nc.vector.tensor_mul(out=xln[:, i, :], in0=xln[:, i, :],
                                         in1=ln1g[:, i * D:(i + 1) * D])
                    nc.vector.tensor_add(out=xln[:, i, :], in0=xln[:, i, :],
                                         in1=ln1b[:, i * D:(i + 1) * D])

                xlnT = ac.tile([P, 4, NTOK], F32R, tag="xlnT")
                for i in range(4):
                    for j in range(4):
                        pst = ps_t.tile([P, P], F32, tag="tps")
                        nc.tensor.transpose(pst[:], xln[:, i, j * P:(j + 1) * P], ident[:])
                        nc.any.tensor_copy(out=xlnT[:, j, i * P:(i + 1) * P], in_=pst[:])

            # --- QKV (fp32 data, fp32r matmuls) ---
            QT = ac.tile([P, 4, NQ], F32R, tag="QT")
            for m in range(4):
                pq = ps_mm.tile([P, NTOK], F32, tag="mm")
                for k in range(4):
                    nc.tensor.matmul(pq[:, :NQ], lhsT=(wq[:, k, m * P:(m + 1) * P]),
                                     rhs=(xlnT[:, k, 0:NQ]), start=(k == 0), stop=(k == 3))
                nc.vector.tensor_scalar(out=QT[:, m, :], in0=pq[:, :NQ],
                                        scalar1=0.125, scalar2=bq[:, m:m + 1],
                                        op0=ALU.mult, op1=ALU.add)
            KT = ac.tile([P, 4, NTOK], F32R, tag="KT")
            for m in range(4):
                pk = ps_mm.tile([P, NTOK], F32, tag="mm")
                for k in range(4):
                    nc.tensor.matmul(pk[:], lhsT=(wk[:, k, m * P:(m + 1) * P]),
                                     rhs=(xlnT[:, k, :]), start=(k == 0), stop=(k == 3))
                nc.vector.tensor_scalar(out=KT[:, m, :], in0=pk[:],
                                        scalar1=bk[:, m:m + 1], scalar2=None,
                                        op0=ALU.add)
            Vh = ac.tile([P, 4, H, HD + 1], F32R, tag="Vh")
            for m in range(4):
                pv = ps_mm.tile([P, NTOK], F32, tag="mm")
                for k in range(4):
                    nc.tensor.matmul(pv[:], lhsT=(xlnT[:, k, m * P:(m + 1) * P]),
                                     rhs=(wv[:, k, :]), start=(k == 0), stop=(k == 3))
                nc.vector.tensor_tensor(
                    out=Vh[:, m, :, 0:HD],
                    in0=pv[:].rearrange("p (h e) -> p h e", h=H),
                    in1=bv_bc.rearrange("p (h e) -> p h e", h=H),
                    op=ALU.add)
            nc.vector.tensor_copy(
                out=Vh[:, :, :, HD:HD + 1],
                in_=ones128[:, 0:32].rearrange("p (a b o) -> p a b o", a=4, b=8))

            # --- per-head attention, software-pipelined in head pairs:
            # scores of the second head are emitted before attnV of the first,
            # so the (in-order) PE queue has work while the first head's
            # softmax Exp runs on the scalar engine.
            OT = ac.tile([HD, H, NQ], F32R, tag="OT")
            for hp in range(H // 2):
                expSs = {}
                for h in (2 * hp, 2 * hp + 1):
                    hb, hc = (h % 2) * HD, h // 2
                    expS = ew.tile([P, 4, NQ], F32R, tag=f"expS{h % 2}")
                    for kc in range(4):
                        pss = ps_mm.tile([P, NTOK], F32, tag="mm")
                        nc.tensor.matmul(pss[:, :NQ],
                                         lhsT=(KT[hb:hb + HD, hc, kc * P:(kc + 1) * P]),
                                         rhs=(QT[hb:hb + HD, hc, :]),
                                         start=True, stop=True)
                        for qc in range(2):
                            nc.scalar.activation(
                                out=expS[:, kc, qc * P:(qc + 1) * P],
                                in_=pss[:, qc * P:(qc + 1) * P], func=AF.Exp,
                                bias=mrow_t[:, kc * 2 + qc:kc * 2 + qc + 1], scale=1.0)
                    expSs[h] = expS
                for h in (2 * hp, 2 * hp + 1):
                    expS = expSs[h]
                    po = ps_o.tile([HD + 1, NQ], F32, tag=f"po{h % 2}")
                    for kc in range(4):
                        nc.tensor.matmul(po[:], lhsT=(Vh[:, kc, h, :]), rhs=(expS[:, kc, :]),
                                         start=(kc == 0), stop=(kc == 3))
                    rden = sm.tile([1, NQ], F32, tag="rden")
                    nc.vector.reciprocal(out=rden[:], in_=po[HD:HD + 1, :])
                    rep = sm.tile([HD, NQ], F32, tag="rep")
                    nc.gpsimd.partition_broadcast(rep[:], rden[:], channels=HD)
                    nc.vector.tensor_mul(out=OT[:, h, :], in0=po[0:HD, :], in1=rep[:])

            # --- Wo + residual ---
            attnT = aw.tile([P, 4, NQ], F32, tag="attnT")
            for m in range(4):
                pw = ps_mm.tile([P, NTOK], F32, tag="mm")
                for h in range(H):
                    nc.tensor.matmul(pw[:, :NQ], lhsT=(woT[:, h, m * P:(m + 1) * P]),
                                     rhs=(OT[:, h, :]), start=(h == 0), stop=(h == H - 1))
                nc.vector.tensor_scalar(out=attnT[:, m, :], in0=pw[:, :NQ],
                                        scalar1=bo[:, m:m + 1], scalar2=None,
                                        op0=ALU.add)

            t_sl = aw.tile([P, 2, D], F32, tag="t_sl")
            for qt in range(2):
                for j in range(4):
                    pst = ps_t.tile([P, P], F32, tag="tps")
                    nc.tensor.transpose(pst[:], attnT[:, j, qt * P:(qt + 1) * P], ident[:])
                    nc.any.tensor_copy(out=t_sl[:, qt, j * P:(j + 1) * P], in_=pst[:])
                nc.vector.tensor_add(out=t_sl[:, qt, :], in0=t_sl[:, qt, :],
                                     in1=x_raw[:, qt, :])
            nc.sync.dma_start(out=t["t_out"][:, :].rearrange("(c p) d -> p c d", p=P),
                              in_=t_sl[:])
            # --- pre-AG: LN2 + router logits for own 2 tiles ---
            for qt in range(2):
                xg2 = aw.tile([P, D], F32, tag="xg2")
                nc.vector.tensor_copy(out=xg2[:], in_=t_sl[:, qt, :])
                _ln_tile(nc, sm, xg2[:], ln2g_o[:, qt * D:(qt + 1) * D],
                         ln2b_o[:, qt * D:(qt + 1) * D], eps_t)
                xgT2 = aw.tile([P, 4, P], F32, tag="xgT2")
                for j in range(4):
                    pst = ps_t.tile([P, P], F32, tag="tps")
                    nc.tensor.transpose(pst[:], xg2[:, j * P:(j + 1) * P], ident[:])
                    nc.any.tensor_copy(out=xgT2[:, j, :], in_=pst[:])
                pl2 = ps_x.tile([P, 8], F32, tag="plog")
                for k in range(4):
                    nc.tensor.matmul(pl2[:], lhsT=xgT2[:, k, :], rhs=wrt[:, qt, k, :],
                                     start=(k == 0), stop=(k == 3))
                lsb = aw.tile([P, 8], F32, tag="lsb")
                nc.vector.tensor_add(out=lsb[:], in0=pl2[:], in1=brt[:, qt, :])
                nc.sync.dma_start(out=agl_in[qt * P:(qt + 1) * P, :], in_=lsb[:])
                xgb = aw.tile([P, D], BF16, tag="xgb")
                nc.vector.tensor_copy(out=xgb[:], in_=xg2[:])
                nc.sync.dma_start(out=agx_in[qt * P:(qt + 1) * P, :], in_=xgb[:])

        if stage < 2:
            return
        if simulate:
            for c in range(NCORES):
                nc.sync.dma_start(out=ag_full[c * NR:(c + 1) * NR, :], in_=ag_in[:, :])
        else:
            nc.gpsimd.collective_compute(
                "AllGather", ALU.bypass,
                replica_groups=[list(range(NCORES))],
                ins=[ag_in[:, :]], outs=[ag_full[:, :]],
            )
        if stage < 3:
            return

        # ================= MoE phase =================
        with contextlib.ExitStack() as mph:
            mc = mph.enter_context(tc.tile_pool(name=R + "mc", bufs=1))
            mw = mph.enter_context(tc.tile_pool(name=R + "mw", bufs=2))
            mr = mph.enter_context(tc.tile_pool(name=R + "mr", bufs=6))

            # unit weights: issued after the collective triggers, so the
            # transfers overlap the AllGathers and don't contend with the
            # attention-phase input DMAs
            for un in units:
                preload_unit(un, wt if un["g"] != "C" else mc, rs4[:, 3:4])

            # --- routing from gathered logits, batched over all 16 tiles ---
            # padded logit columns (E..8) arrive as -1e9 (host pads brt), so
            # every op below can run on the full [P, 16, 8] block at once.
            lall = mc.tile([P, 16, 8], F32, tag="lall")
            for qv in range(2):
                nc.sync.dma_start(
                    out=lall[:].rearrange("p (c q) e -> p c q e", c=8, q=2)[:, :, qv, :],
                    in_=log_full[:, qv * P:(qv + 1) * P, :].rearrange("c p e -> p c e"))

            def bcl(ap, n):
                return bass.AP(tensor=ap.tensor, offset=ap.offset,
                               ap=list(ap.ap) + [[0, n]])

            def bcm(ap, dims):
                a = list(ap.ap)
                return bass.AP(tensor=ap.tensor, offset=ap.offset,
                               ap=a[:-1] + [[0, n] for n in dims] + [a[-1]])

            def v4(ap):
                return ap.rearrange("p (b c) e -> p b c e", c=4)

            m1a = mc.tile([P, 16], F32, tag="m1a")
            nc.vector.reduce_max(out=m1a[:], in_=lall[:], axis=mybir.AxisListType.X)
            masks1 = mc.tile([P, 16, 8], F32, tag="masks1")
            nc.vector.tensor_tensor(out=masks1[:], in0=lall[:], in1=bcl(m1a[:], 8),
                                    op=ALU.is_equal)
            tmp16 = mr.tile([P, 16, 8], F32, tag="tmp16")
            nc.vector.tensor_scalar(out=tmp16[:], in0=masks1[:], scalar1=-1e9,
                                    op0=ALU.mult, scalar2=None)
            nc.vector.tensor_add(out=tmp16[:], in0=tmp16[:], in1=lall[:])
            m2a = mc.tile([P, 16], F32, tag="m2a")
            nc.vector.reduce_max(out=m2a[:], in_=tmp16[:], axis=mybir.AxisListType.X)
            masks2 = mc.tile([P, 16, 8], F32, tag="masks2")
            nc.vector.tensor_tensor(out=masks2[:], in0=tmp16[:], in1=bcl(m2a[:], 8),
                                    op=ALU.is_equal)
            # maskor: top1 for C tiles (chunks 1, 2), top1+top2 for A/B
            mor16 = mc.tile([P, 16, 8], F32, tag="mor16")
            nc.vector.tensor_copy(out=mor16[:], in_=masks1[:])
            for ci in (0, 3):
                nc.vector.tensor_add(out=v4(mor16[:])[:, :, ci, :],
                                     in0=v4(mor16[:])[:, :, ci, :],
                                     in1=v4(masks2[:])[:, :, ci, :])

            # gates: A/B renormalized top-2 via sigmoid; C is the top-1 mask
            d12 = mr.tile([P, 16], F32, tag="d12")
            nc.vector.tensor_tensor(out=d12[:], in0=m1a[:], in1=m2a[:], op=ALU.subtract)
            g1v = mr.tile([P, 16], F32, tag="g1v")
            nc.scalar.activation(out=g1v[:], in_=d12[:], func=AF.Sigmoid)
            g2v = mr.tile([P, 16], F32, tag="g2v")
            nc.vector.tensor_scalar(out=g2v[:], in0=g1v[:], scalar1=-1.0,
                                    scalar2=1.0, op0=ALU.mult, op1=ALU.add)
            gAB = mr.tile([P, 16, 8], F32, tag="gAB")
            nc.vector.tensor_tensor(out=gAB[:], in0=masks1[:], in1=bcl(g1v[:], 8),
                                    op=ALU.mult)
            tmp2 = mr.tile([P, 16, 8], F32, tag="tmp2")
            nc.vector.tensor_tensor(out=tmp2[:], in0=masks2[:], in1=bcl(g2v[:], 8),
                                    op=ALU.mult)
            nc.vector.tensor_add(out=gAB[:], in0=gAB[:], in1=tmp2[:])
            nc.sync.dma_start(out=t["gates_a"][:, :].rearrange("(b p) e -> p b e", p=P),
                              in_=v4(gAB[:])[:, :, 0, :EA])
            nc.sync.dma_start(out=t["gates_b"][:, :].rearrange("(b p) e -> p b e", p=P),
                              in_=v4(gAB[:])[:, :, 3, :EB])
            for ci in (1, 2):
                nc.sync.dma_start(
                    out=t["gates_c"][:, :].rearrange("(b c p) e -> p b c e", p=P,
                                                     c=2)[:, :, ci - 1, :],
                    in_=v4(masks1[:])[:, :, ci, :EC])

            # --- positions: within-tile prefix (LT) + per-group tile-cumulative ---
            morf = mor16[:].rearrange("p a e -> p (a e)")
            pref = ps_mm.tile([P, NTOK], F32, tag="mm")
            nc.tensor.matmul(pref[:, :128], lhsT=LT[:], rhs=morf, start=True, stop=True)
            tots = ps_mm.tile([P, NTOK], F32, tag="mm")
            nc.tensor.matmul(tots[:, :128], lhsT=ones128[:], rhs=morf, start=True, stop=True)
            totsb = mr.tile([1, 128], F32, tag="totsb")
            nc.vector.tensor_copy(out=totsb[:], in_=tots[0:1, :128])
            cumrow = mr.tile([1, 128], F32, tag="cumrow")
            nc.vector.memset(cumrow[:], 0.0)
            for g in ("A", "C", "B"):
                gt = GROUP_TILES[g]
                for idx in range(1, len(gt)):
                    a, prv = gt[idx] * 8, gt[idx - 1] * 8
                    nc.vector.tensor_add(out=cumrow[:, a:a + 8],
                                         in0=cumrow[:, prv:prv + 8],
                                         in1=totsb[:, prv:prv + 8])
            posall16 = mc.tile([P, 16, 8], F32, tag="posall16")
            pflat = posall16[:].rearrange("p a e -> p (a e)")
            nc.vector.tensor_copy(out=pflat, in_=pref[:, :128])
            cumb = ps_mm.tile([P, NTOK], F32, tag="mm")
            nc.tensor.matmul(cumb[:, :128], lhsT=ones128[0:1, :], rhs=cumrow[:],
                             start=True, stop=True)
            nc.vector.tensor_add(out=pflat, in0=pflat, in1=cumb[:, :128])

            if stage < 4:
                return
            # --- expert units (matmul-based compaction) ---
            # (A/B weights were preloaded during attention; xbf load is
            # issued only now so the sync queue isn't blocked on the token
            # AG while routing runs)
            xbf = mc.tile([P, 16, D], BF16, tag="xbf")
            for qv in range(2):
                nc.sync.dma_start(
                    out=xbf[:].rearrange("p (c q) d -> p c q d", c=8, q=2)[:, :, qv, :],
                    in_=xbf_full[:, qv * P:(qv + 1) * P, :].rearrange("c p d -> p c d"))

            # the unit's expert id arrives as data: host encodes it by
            # pre-multiplying a one-hot (E,) selection into... simpler: the
            # expert id only affects WHICH pos/mask column is used. That
            # must be uniform across cores -> pass per-unit expert one-hot
            # as an input row and select the column via a tiny matmul.
            for un in units:
                g, cap, F, tg = un["g"], un["cap"], un["F"], un["tg"]
                E = GROUPS[g]["E"]
                gtiles = GROUP_TILES[g]
                ntiles = len(gtiles)
                fch = F // P
                w1, w2, b1, b2, eoh = (un["w1_sb"], un["w2_sb"], un["b1_sb"],
                                       un["b2_sb"], un["eoh_sb"])
                cls = "ab" if F == FA else "c"


                # G tiles: G[t, j] = (pos[t, e] == j) * maskor[t, e]
                # batched per-unit expert-column selection via one-hot dot
                pos_u = mr.tile([P, ntiles], F32, tag=f"pos_u_{cls}", name="pos_u")
                msk_u = mr.tile([P, ntiles], F32, tag=f"msk_u_{cls}", name="msk_u")
                if g == "C":
                    pv = v4(posall16[:])[:, :, 1:3, :]
                    mv = v4(mor16[:])[:, :, 1:3, :]
                    eb = bcm(eoh[:, :], (4, 2))
                    po_ = pos_u[:].rearrange("p (b c) -> p b c", c=2)
                    mo_ = msk_u[:].rearrange("p (b c) -> p b c", c=2)
                else:
                    ci = 0 if g == "A" else 3
                    pv = v4(posall16[:])[:, :, ci, :]
                    mv = v4(mor16[:])[:, :, ci, :]
                    eb = bcm(eoh[:, :], (4,))
                    po_, mo_ = pos_u[:], msk_u[:]
                tsel = mr.tile([P, ntiles, 8], F32, tag=f"tsel_{cls}", name="tsel")
                tv = (tsel[:].rearrange("p (b c) e -> p b c e", c=2)
                      if g == "C" else tsel[:])
                nc.vector.tensor_tensor(out=tv, in0=pv, in1=eb, op=ALU.mult)
                nc.vector.reduce_sum(out=po_, in_=tv, axis=mybir.AxisListType.X)
                nc.vector.tensor_tensor(out=tv, in0=mv, in1=eb, op=ALU.mult)
                nc.vector.reduce_sum(out=mo_, in_=tv, axis=mybir.AxisListType.X)
                Gts = []
                for ti in range(ntiles):
                    Gt = mw.tile([P, cap], BF16, tag=f"Gt{ti}_{cls}", name=f"Gt{ti}_{cls}")
                    nc.vector.tensor_scalar(out=Gt[:], in0=iotaCap[:, :cap],
                                            scalar1=pos_u[:, ti:ti + 1],
                                            op0=ALU.is_equal, scalar2=None)
                    nc.vector.tensor_scalar(out=Gt[:], in0=Gt[:],
                                            scalar1=msk_u[:, ti:ti + 1],
                                            op0=ALU.mult, scalar2=None)
                    Gts.append(Gt)

                # compact^T (feature-major, bf16): [128d x cap] per d-chunk
                xT = mw.tile([P, 4, cap], BF16, tag=f"xTg_{cls}", name=f"xTg_{cls}")
                for m in range(4):
                    pc = ps_mm.tile([P, NTOK], F32, tag="mm")
                    for ti in range(ntiles):
                        nc.tensor.matmul(pc[:, :cap],
                                         lhsT=xbf[:, gtiles[ti], m * P:(m + 1) * P],
                                         rhs=Gts[ti][:],
                                         start=(ti == 0), stop=(ti == ntiles - 1))
                    nc.any.tensor_copy(out=xT[:, m, :], in_=pc[:, :cap])
                if stage == 5:
                    red = sm.tile([P, 1], F32, tag="dbg_red", name="dbg_red")
                    nc.vector.reduce_sum(out=red[:], in_=xT[:], axis=mybir.AxisListType.XY)
                    nc.sync.dma_start(out=un["yout"][0:P, 0:1], in_=red[:])
                    continue

                # FFN
                hT = mw.tile([P, fch, cap], BF16, tag=f"hT_{cls}", name=f"hT_{cls}")
                for m in range(fch):
                    ph = ps_mm.tile([P, NTOK], F32, tag="mm")
                    for k in range(4):
                        nc.tensor.matmul(ph[:, :cap], lhsT=w1[:, k, m * P:(m + 1) * P],
                                         rhs=xT[:, k, :], start=(k == 0), stop=(k == 3))
                    nc.scalar.activation(out=hT[:, m, :], in_=ph[:, :cap],
                                         func=AF.Gelu_apprx_tanh,
                                         bias=b1[:, m:m + 1], scale=1.0)
                if stage == 6:
                    red = sm.tile([P, 1], F32, tag="dbg_red", name="dbg_red")
                    nc.vector.reduce_sum(out=red[:], in_=hT[:], axis=mybir.AxisListType.XY)
                    nc.sync.dma_start(out=un["yout"][0:P, 0:1], in_=red[:])
                    continue
                y = mw.tile([P, 4, cap], BF16, tag=f"y_{cls}", name=f"y_{cls}")
                for m in range(4):
                    py = ps_mm.tile([P, NTOK], F32, tag="mm")
                    for k in range(fch):
                        nc.tensor.matmul(py[:, :cap], lhsT=w2[:, k, m * P:(m + 1) * P],
                                         rhs=hT[:, k, :], start=(k == 0), stop=(k == fch - 1))
                    nc.scalar.activation(out=y[:, m, :], in_=py[:, :cap], func=AF.Identity,
                                         bias=b2[:, m:m + 1], scale=1.0)
                nc.gpsimd.dma_start(out=un["yout"][:, :].rearrange("p (o c) -> p o c", o=4),
                                  in_=y[:])


@functools.lru_cache(maxsize=8)
def _build(stage=4, reps=1, simulate=False):
    nc = bacc.Bacc("TRN2", target_bir_lowering=False, debug=False,
                   num_devices=1 if simulate else NCORES,
                   enable_asserts=False)
    t = _declare_io(nc)
    with tile.TileContext(nc) as tc:
        for r in range(reps):
            _body(nc, tc, t, stage=stage, rep=r, simulate=simulate)
    nc.compile()
    return nc


# --------------------------------------------------------------------------
# host side
# --------------------------------------------------------------------------

def _seg_mask():
    seg = np.concatenate([np.zeros(128), np.ones(256), 2 * np.ones(128)])
    allowed = seg[None, :] <= seg[:, None]  # (q, k)
    return np.where(allowed, 0.0, NEG_MASK).astype(np.float32)


def _host_inputs(inputs):
    """Build the 8 per-core input maps."""
    f32 = np.float32
    bf = ml_dtypes.bfloat16
    tokens_A = np.asarray(inputs["tokens_A"], f32)
    tokens_B = np.asarray(inputs["tokens_B"], f32)
    tokens_C = np.asarray(inputs["tokens_C"], f32)
    ln1_g = np.asarray(inputs["ln1_g"], f32); ln1_b = np.asarray(inputs["ln1_b"], f32)
    ln2_g = np.asarray(inputs["ln2_g"], f32); ln2_b = np.asarray(inputs["ln2_b"], f32)
    Wqkv = np.asarray(inputs["Wqkv"], f32); bqkv = np.asarray(inputs["bqkv"], f32)
    Wo = np.asarray(inputs["Wo"], f32); bo = np.asarray(inputs["bo"], f32)

    X = np.concatenate([tokens_A, tokens_C, tokens_B], axis=1)  # (B, 512, D)

    def pb(x):
        x = np.asarray(x, f32)
        return np.ascontiguousarray(np.broadcast_to(x.reshape(1, -1), (128, x.size)))

    def tile_rows(x, p=128):
        # (o*p, X) -> (p, o*X) with [p, o, X] layout (per-partition contiguous)
        x = np.asarray(x)
        o = x.shape[0] // p
        return np.ascontiguousarray(
            x.reshape(o, p, -1).transpose(1, 0, 2).reshape(p, -1))

    wr_pad = {}
    br_pad = {}
    for g, wr_k, br_k in (("A", "Wr_A", "br_A"), ("B", "Wr_B", "br_B"), ("C", "Wr_C", "br_C")):
        E = {"A": EA, "B": EB, "C": EC}[g]
        wrp = np.zeros((D, 8), f32); wrp[:, :E] = np.asarray(inputs[wr_k], f32)
        brp = np.full(8, -1e9, f32); brp[:E] = np.asarray(inputs[br_k], f32)
        wr_pad[g] = wrp; br_pad[g] = brp

    attn_w = np.concatenate([tile_rows(Wqkv.T[:, 0:D]),
                             tile_rows(Wqkv.T[:, D:2 * D]),
                             tile_rows(Wqkv.T[:, 2 * D:3 * D])], axis=1)
    bq_r = np.ascontiguousarray((bqkv[:512] * 0.125).reshape(4, 128).T)
    bk_r = np.ascontiguousarray(bqkv[512:1024].reshape(4, 128).T)
    bo_r = np.ascontiguousarray(bo.reshape(4, 128).T)
    bv_row = bqkv[1024:].reshape(1, D)
    base = dict(attn_w=attn_w, woT=tile_rows(Wo.T, p=64))
    W1_A = np.asarray(inputs["W1_A"], f32); W2_A = np.asarray(inputs["W2_A"], f32)
    b1_A = np.asarray(inputs["b1_A"], f32); b2_A = np.asarray(inputs["b2_A"], f32)
    W1_B = np.asarray(inputs["W1_B"], f32); W2_B = np.asarray(inputs["W2_B"], f32)
    b1_B = np.asarray(inputs["b1_B"], f32); b2_B = np.asarray(inputs["b2_B"], f32)
    W1_C = np.asarray(inputs["W1_C"], f32); W2_C = np.asarray(inputs["W2_C"], f32)
    b1_C = np.asarray(inputs["b1_C"], f32); b2_C = np.asarray(inputs["b2_C"], f32)

    chunk_groups = [["A", "C", "C", "B"], ["C", "B", "A", "C"]]
    gidx = {"A": 0, "C": 1, "B": 2}

    seg_of = {"A": 0, "C": 1, "B": 2}
    in_maps = []
    for c in range(NCORES):
        b, s = c // 2, c % 2
        xb = np.roll(X[b], -s * 256, axis=0)  # queries at rows 0:256
        # chunk-level mask table in rolled coords: mrow[kc*2+qc]
        segs = [seg_of[g] for g in chunk_groups[s]]
        mrow = np.zeros((1, 8), f32)
        for kc in range(4):
            for qc in range(2):
                mrow[0, kc * 2 + qc] = 0.0 if segs[kc] <= segs[qc] else NEG_MASK
        g1c = np.stack([ln1_g[gidx[g]] for g in chunk_groups[s]]).reshape(1, 4 * D)
        b1c = np.stack([ln1_b[gidx[g]] for g in chunk_groups[s]]).reshape(1, 4 * D)
        own_groups = chunk_groups[s][:2]
        g2c = np.stack([ln2_g[gidx[g]] for g in own_groups]).reshape(1, 2 * D)
        b2c = np.stack([ln2_b[gidx[g]] for g in own_groups]).reshape(1, 2 * D)
        brow = np.concatenate([g1c, b1c, g2c, b2c, bv_row.astype(f32), mrow],
                              axis=1)
        # per-own-tile padded router weights: (P, 2*4*8) / (P, 2*8)
        wrt = np.stack([tile_rows(wr_pad[g]).reshape(128, 4, 8) for g in own_groups],
                       axis=1).reshape(128, 2 * 4 * 8)
        brt = np.stack([np.broadcast_to(br_pad[g], (128, 8)) for g in own_groups],
                       axis=1).reshape(128, 2 * 8)

        eA = c // 2; fA = c % 2
        eB = c // 2; fB = c % 2
        cu = [(3 * c + u) // 4 for u in range(3)]   # C experts per unit
        cq = [(3 * c + u) % 4 for u in range(3)]    # C quarter per unit
        smalls = np.concatenate([bq_r, bk_r, bo_r, wrt.reshape(128, 64),
                                 brt.reshape(128, 16)], axis=1)
        m = dict(base)
        m.update(
            x_tok=np.ascontiguousarray(xb),
            brow=brow, smalls=smalls,
            w1_a=tile_rows(W1_A[eA][:, fA * FA:(fA + 1) * FA]).astype(bf),
            w2_a=tile_rows(W2_A[eA][fA * FA:(fA + 1) * FA, :]).astype(bf),
            b1_a=np.ascontiguousarray(b1_A[eA][fA * FA:(fA + 1) * FA].reshape(FA // P, P).T),
            b2_a=np.ascontiguousarray((b2_A[eA] if fA == 0 else np.zeros(D, f32)).reshape(4, P).T),
            w1_b=tile_rows(W1_B[eB][:, fB * FA:(fB + 1) * FA]).astype(bf),
            w2_b=tile_rows(W2_B[eB][fB * FA:(fB + 1) * FA, :]).astype(bf),
            b1_b=np.ascontiguousarray(b1_B[eB][fB * FA:(fB + 1) * FA].reshape(FA // P, P).T),
            b2_b=np.ascontiguousarray((b2_B[eB] if fB == 0 else np.zeros(D, f32)).reshape(4, P).T),
            w1_c=np.stack([tile_rows(W1_C[cu[u]][:, cq[u] * FC:(cq[u] + 1) * FC]) for u in range(3)]).astype(bf),
            w2_c=np.stack([tile_rows(W2_C[cu[u]][cq[u] * FC:(cq[u] + 1) * FC, :]) for u in range(3)]).astype(bf),
            b1_c=np.stack([b1_C[cu[u]][cq[u] * FC:(cq[u] + 1) * FC].reshape(FC // P, P).T for u in range(3)]),
            b2_c=np.stack([(b2_C[cu[u]] if cq[u] == 0 else np.zeros(D, f32)).reshape(4, P).T for u in range(3)]),
            eoh_a=pb(np.eye(EA, dtype=f32)[eA]),
            eoh_b=pb(np.eye(EB, dtype=f32)[eB]),
            eoh_c0=pb(np.eye(EC, dtype=f32)[cu[0]]),
            eoh_c1=pb(np.eye(EC, dtype=f32)[cu[1]]),
            eoh_c2=pb(np.eye(EC, dtype=f32)[cu[2]]),
        )
        in_maps.append({k: np.ascontiguousarray(v) for k, v in m.items()})
    return in_maps


def _combine(results):
    """Host combine: out = t_full + sum gate * y, slots derived from gates."""
    f32 = np.float32
    t_full = np.concatenate([results[c]["t_out"] for c in range(NCORES)], axis=0)
    out = t_full.astype(f32).copy()

    def apply_unit(ytab, gates, e, cap, goff, gwidth):
        sel = gates[:, e] > 0
        gr = np.nonzero(sel)[0][:cap]
        if gr.size == 0:
            return
        rows = (gr // gwidth) * NTOK + goff + (gr % gwidth)
        gate = gates[gr, e]
        # ytab (128, 4*cap) with [p, o, cap] layout -> y^T (512, cap)
        yt = ytab.reshape(P, 4, cap).transpose(1, 0, 2).reshape(4 * P, cap)
        y = yt.T[:gr.size].astype(f32)
        np.add.at(out, rows, gate[:, None].astype(f32) * y)

    for c in range(NCORES):
        r = results[c]
        apply_unit(r["ya_t"], r["gates_a"], c // 2, CAP_A, 0, 128)
        apply_unit(r["yb_t"], r["gates_b"], c // 2, CAP_B, 384, 128)
        for u in range(3):
            cu = (3 * c + u) // 4
            apply_unit(r["yc_t"][u], r["gates_c"], cu, CAP_C, 128, 256)

    return out.reshape(BATCH, NTOK, D)


_LAST_RESULTS = None
_EXEC_CACHE = {}


def _get_exec(stage=4, reps=1):
    """Build (once) a cached jitted shard_map executable for the NEFF."""
    if (stage, reps) in _EXEC_CACHE:
        return _EXEC_CACHE[(stage, reps)]
    import jax
    from jax.sharding import Mesh, PartitionSpec
    from jax.experimental.shard_map import shard_map
    from concourse.bass2jax import install_neuronx_cc_hook, _bass_exec_p
    import concourse.mybir as _mybir

    nc = _build(stage, reps)
    install_neuronx_cc_hook()
    in_names, out_names, out_avals, zero_outs = [], [], [], []
    assert nc.dbg_addr is None
    partition_name = nc.partition_id_tensor.name if nc.partition_id_tensor else None
    for alloc in nc.m.functions[0].allocations:
        if not isinstance(alloc, _mybir.MemoryLocationSet):
            continue
        name = alloc.memorylocations[0].name
        if alloc.kind == "ExternalInput":
            if name != partition_name:
                in_names.append(name)
        elif alloc.kind == "ExternalOutput":
            shape = tuple(alloc.tensor_shape)
            dtype = _mybir.dt.np(alloc.dtype)
            out_names.append(name)
            out_avals.append(jax.core.ShapedArray(shape, dtype))
            zero_outs.append(np.zeros(shape, dtype))
    n_params = len(in_names)
    all_names = in_names + out_names
    if partition_name is not None:
        all_names = all_names + [partition_name]

    def _fn(*args):
        from concourse.bass2jax import partition_id_tensor as _pid
        operands = list(args)
        if partition_name is not None:
            operands.append(_pid())
        outs = _bass_exec_p.bind(
            *operands,
            out_avals=tuple(out_avals),
            in_names=tuple(all_names),
            out_names=tuple(out_names),
            lowering_input_output_aliases=(),
            sim_require_finite=True,
            sim_require_nnan=True,
            nc=nc,
        )
        return tuple(outs)

    devices = jax.devices()[:NCORES]
    mesh = Mesh(np.asarray(devices), ("core",))
    nin = n_params + len(out_names)
    sharded = jax.jit(
        shard_map(_fn, mesh=mesh, in_specs=(PartitionSpec("core"),) * nin,
                  out_specs=(PartitionSpec("core"),) * len(out_names),
                  check_rep=False),
        keep_unused=True)
    _EXEC_CACHE[(stage, reps)] = dict(
        fn=sharded, in_names=in_names, out_names=out_names,
        out_avals=out_avals, zero_outs=zero_outs, mesh=mesh, stage=(stage, reps))
    return _EXEC_CACHE[(stage, reps)]


def _concat_inputs(in_maps, ex):
    concat = [np.concatenate([np.asarray(in_maps[c][nm]) for c in range(NCORES)], axis=0)
              for nm in ex["in_names"]]
    zeros = [np.zeros((NCORES * z.shape[0], *z.shape[1:]), z.dtype)
             for z in ex["zero_outs"]]
    return concat + zeros


def _run(in_maps):
    ex = _get_exec()
    args = _concat_inputs(in_maps, ex)
    out_arrs = ex["fn"](*args)
    results = []
    for c in range(NCORES):
        r = {}
        for i, nm in enumerate(ex["out_names"]):
            shp = ex["out_avals"][i].shape
            r[nm] = np.asarray(out_arrs[i]).reshape(NCORES, *shp)[c]
        results.append(r)
    return results


def kernel(**inputs):
    global _LAST_RESULTS
    in_maps = _host_inputs(inputs)
    results = _run(in_maps)
    _LAST_RESULTS = results
    return _combine(results)



# revision 44
# speedup vs baseline: 1.0461x; 1.0461x over previous
"""Trainium2 Bass kernel for nn_MoEBlock_78288663872291 (moe_routing).

Sharding across 8 NeuronCores (single SPMD NEFF, per-core differences are
input *data* only):
  - Attention: core c handles batch c//2, query-half c%2. Host rolls each
    batch's 512-token [A|C|B] sequence so this core's 256 queries are always
    rows 0:256 (keeps the program uniform). KV projection is computed for
    the full 512 tokens (duplicated across the 2 cores of a batch). All
    attention matmuls run as float32r (full fp32 data, 1 cycle/row on the
    PE for moving dims >= 256, vs 4 for plain fp32) - the MoE top-k routing
    is numerically sensitive, and fp32r keeps it bit-stable enough.
    The directed [A|C|B] mask is a per-(key-chunk, query-chunk) 8-entry
    table fused into the softmax Exp as an activation bias. LN params /
    biases arrive as single rows and are broadcast on-chip.
  - ONE fused AllGather: each core contributes 256 bf16 LN2'd token rows
    plus 8 rows carrying the (D,8) f32 router logits bitcast into the bf16
    buffer. A single collective ~halves the per-op ncfw latency floor paid
    vs two separate AGs at LNC1 x8 ranks.
  - MoE expert-parallel: every core redundantly computes routing for all
    16 tiles in one batched pass ([P,16,8] top-1/top-2 masks, sigmoid
    gates, prefix-position counts via two 128-wide matmuls), then runs its
    expert F-slices on capacity-compacted tokens (compaction via one-hot
    G matmuls):
      unit A: expert c//2, F columns (c%2)*1024..+1024 of W1/W2
      unit B: same
      units C x3: quarter-F slices; global quarter q = 3c+u -> expert q//4,
                  F columns (q%4)*512..+512
    Expert weights (bf16) are preloaded on the sync DMA ring, gated behind
    LN1 completion so their transfers never contend with the attention-
    critical input loads.
  - Host combine: out = t_full + sum over (core, unit) of
      gate[token, expert] * y_unit[j] (y tables in bf16) scattered by the
      device-produced gate tables.
"""

import functools
import numpy as np
import ml_dtypes

import concourse.bass as bass
import concourse.bacc as bacc
import concourse.tile as tile
import concourse.mybir as mybir
from concourse.masks import make_identity

F32 = mybir.dt.float32
F32R = mybir.dt.float32r
BF16 = mybir.dt.bfloat16
FP8 = mybir.dt.float8e4
I32 = mybir.dt.int32
AF = mybir.ActivationFunctionType
ALU = mybir.AluOpType

NCORES = 8
P = 128
D = 512
NTOK = 512
NQ = 256
H = 8
HD = 64
BATCH = 4
NGLOBAL = BATCH * NTOK  # 2048

EA, EB, EC = 4, 4, 6
CAP_A, CAP_B, CAP_C = 384, 384, 256
FA, FC = 1024, 512
SENTINEL = float(1 << 20)
NEG_MASK = -30.0
EPS = 1e-5

# groups within a batch's [A|C|B] row block
GROUPS = {
    "A": dict(E=EA, K=2, cap=CAP_A, off=0, width=128),
    "C": dict(E=EC, K=1, cap=CAP_C, off=128, width=256),
    "B": dict(E=EB, K=2, cap=CAP_B, off=384, width=128),
}
# global 128-row tile i of t_full: batch i//4, chunk i%4 -> group
CHUNK_GROUP = ["A", "C", "C", "B"]
GROUP_TILES = {
    "A": [4 * b + 0 for b in range(BATCH)],
    "C": [4 * b + c for b in range(BATCH) for c in (1, 2)],
    "B": [4 * b + 3 for b in range(BATCH)],
}


# --------------------------------------------------------------------------
# device kernel
# --------------------------------------------------------------------------

def _declare_io(nc):
    t = {}
    def I(name, shape, dt=F32):
        t[name] = nc.dram_tensor(name, shape, dt, kind="ExternalInput")
    def O(name, shape, dt=F32):
        t[name] = nc.dram_tensor(name, shape, dt, kind="ExternalOutput")
    I("x_tok", [NTOK, D])            # rolled tokens of this core's batch
    # brow: [g1c(4D) | b1c(4D) | g2c(2D) | b2c(2D) | bv(D) | mrow(8)]
    I("brow", [1, 13 * D + 8])
    I("attn_w", [P, 12 * D], F32R)   # [wq | wk | wv] pre-tiled
    I("smalls", [P, 92])             # [bq(4) | bk(4) | bo(4) | wrt(64) | brt(16)]
    I("woT", [HD, H * D], F32R)
    I("w1_a", [P, 4 * FA], BF16); I("w2_a", [P, (FA // P) * D], BF16)
    I("b1_a", [P, FA // P]); I("b2_a", [P, 4])
    I("w1_b", [P, 4 * FA], BF16); I("w2_b", [P, (FA // P) * D], BF16)
    I("b1_b", [P, FA // P]); I("b2_b", [P, 4])
    I("w1_c", [3, P, 4 * FC], BF16); I("w2_c", [3, P, (FC // P) * D], BF16)
    I("b1_c", [3, P, FC // P]); I("b2_c", [3, P, 4])
    I("eoh_a", [P, EA]); I("eoh_b", [P, EB])
    I("eoh_c0", [P, EC]); I("eoh_c1", [P, EC]); I("eoh_c2", [P, EC])
    O("t_out", [NQ, D])
    O("ya_t", [P, 4 * CAP_A], BF16); O("yb_t", [P, 4 * CAP_B], BF16)
    O("yc_t", [3, P, 4 * CAP_C], BF16)
    O("gates_a", [BATCH * 128, EA])
    O("gates_b", [BATCH * 128, EB])
    O("gates_c", [BATCH * 256, EC])
    return t


def _ln_tile(nc, sm, x_sl, g_t, b_t, eps_t):
    """In-place layernorm over the free axis of x_sl (128, D)."""
    stats = sm.tile([P, nc.vector.BN_STATS_DIM], F32, tag="ln_stats")
    nc.vector.bn_stats(out=stats[:], in_=x_sl)
    mv = sm.tile([P, nc.vector.BN_AGGR_DIM], F32, tag="ln_mv")
    nc.vector.bn_aggr(out=mv[:], in_=stats[:])
    nc.scalar.activation(out=mv[:, 1:2], in_=mv[:, 1:2], func=AF.Sqrt,
                         bias=eps_t[:], scale=1.0)
    nc.vector.reciprocal(out=mv[:, 1:2], in_=mv[:, 1:2])
    nc.vector.tensor_scalar(out=x_sl, in0=x_sl, scalar1=mv[:, 0:1],
                            scalar2=mv[:, 1:2], op0=ALU.subtract, op1=ALU.mult)
    nc.vector.tensor_mul(out=x_sl, in0=x_sl, in1=g_t)
    nc.vector.tensor_add(out=x_sl, in0=x_sl, in1=b_t)


def _body(nc, tc, t, stage=4, rep=0, simulate=False):
    import contextlib
    R = f"r{rep}_"
    with contextlib.ExitStack() as top:
        const = top.enter_context(tc.tile_pool(name=R + "const", bufs=1))
        wt = top.enter_context(tc.tile_pool(name=R + "wt", bufs=1))
        sm = top.enter_context(tc.tile_pool(name=R + "sm", bufs=8))
        dram = top.enter_context(tc.tile_pool(name=R + "dram", bufs=1, space="DRAM"))
        ps_mm = top.enter_context(tc.tile_pool(name=R + "ps_mm", bufs=2, space="PSUM"))
        ps_t = top.enter_context(tc.tile_pool(name=R + "ps_t", bufs=2, space="PSUM"))
        ps_o = top.enter_context(tc.tile_pool(name=R + "ps_o", bufs=1, space="PSUM"))
        ps_x = top.enter_context(tc.tile_pool(name=R + "ps_x", bufs=1, space="PSUM"))

        ident = const.tile([P, P], F32, tag="ident")
        make_identity(nc, ident[:])
        eps_t = const.tile([P, 1], F32, tag="eps")
        nc.vector.memset(eps_t[:], EPS)
        ones128 = const.tile([P, P], F32, tag="ones128")
        nc.vector.memset(ones128[:], 1.0)
        # strict-lower-triangular (LT[p, f] = 1.0 iff p < f) for prefix counts
        LT = const.tile([P, P], F32, tag="LT")
        nc.gpsimd.memset(LT[:], 1.0)
        nc.gpsimd.affine_select(out=LT[:], in_=LT[:], compare_op=ALU.is_gt,
                                fill=0.0, base=0, pattern=[[1, P]],
                                channel_multiplier=-1)
        # iota over capacity slots (free axis), fp32 exact ints
        iotaCap_i = const.tile([P, CAP_A], I32, tag="iotaCap_i")
        nc.gpsimd.iota(iotaCap_i[:], pattern=[[1, CAP_A]], base=0, channel_multiplier=0)
        iotaCap = const.tile([P, CAP_A], F32, tag="iotaCap")
        nc.vector.tensor_copy(out=iotaCap[:], in_=iotaCap_i[:])

        def bcast(src_row, tag, w=D):
            tl = const.tile([P, w], F32, tag=tag, name=tag)
            nc.gpsimd.dma_start(out=tl[:], in_=src_row.to_broadcast((P, w)))
            return tl

        # dram scratch (shared across reps so repetitions serialize)
        # ag_in rows 0:256 = bf16 tokens; rows 256:264 = f32 logits (bitcast)
        NR = NQ + 8
        if "ag_in" not in t:
            t["ag_in"] = nc.dram_tensor("ag_in", [NR, D], BF16, kind="Internal")
            t["ag_full"] = nc.dram_tensor("ag_full", [NCORES * NR, D], BF16,
                                          kind="Internal", addr_space="Shared")
        ag_in, ag_full = t["ag_in"], t["ag_full"]
        agx_in = ag_in[0:NQ, :]
        # [256, 8] f32 view of the logit rows (token-major)
        agl_in = ag_in[NQ:NR, :].bitcast(F32).rearrange("a (p e) -> (a p) e", e=8)
        ag3 = ag_full[:, :].rearrange("(c r) d -> c r d", r=NR)
        xbf_full = ag3[:, 0:NQ, :]
        log_full = ag3[:, NQ:NR, :].bitcast(F32).rearrange(
            "c a (p e) -> c (a p) e", e=8)

        if stage == 0:
            z = sm.tile([P, D], F32, tag="z0", name="z0")
            nc.sync.dma_start(out=z[:], in_=t["x_tok"][0:P, :])
            nc.sync.dma_start(out=t["t_out"][0:P, :], in_=z[:])
            return

        # ================= attention phase =================
        with contextlib.ExitStack() as aph:
            ac = aph.enter_context(tc.tile_pool(name=R + "ac", bufs=1))
            aw = aph.enter_context(tc.tile_pool(name=R + "aw", bufs=2))
            ew = aph.enter_context(tc.tile_pool(name=R + "ew", bufs=1))

            # critical-path input DMAs first, merged to minimize
            # descriptor-generation serialization on the sync engine
            x_raw = ac.tile([P, 4, D], F32, tag="x_raw")
            nc.sync.dma_start(out=x_raw[:], in_=t["x_tok"][:, :].rearrange("(c p) d -> p c d", p=P))
            attw = ac.tile([P, 3, 4, D], F32R, tag="attw")
            nc.sync.dma_start(out=attw[:], in_=t["attn_w"][:, :].rearrange("p (w o r) -> p w o r", w=3, o=4))
            woT = ac.tile([HD, H, D], F32R, tag="woT")
            nc.sync.dma_start(out=woT[:], in_=t["woT"][:, :].rearrange("p (h d) -> p h d", h=H))
            smt = ac.tile([P, 92], F32, tag="smt")
            nc.sync.dma_start(out=smt[:], in_=t["smalls"][:, :])
            brow_t = ac.tile([P, 13 * D + 8], F32, tag="brow_t")
            nc.gpsimd.dma_start(out=brow_t[:],
                                in_=t["brow"][0:1, :].to_broadcast((P, 13 * D + 8)))
            wq, wk, wv = attw[:][:, 0], attw[:][:, 1], attw[:][:, 2]
            bq, bk, bo = smt[:][:, 0:4], smt[:][:, 4:8], smt[:][:, 8:12]
            wrt = smt[:][:, 12:76].rearrange("p (c k e) -> p c k e", c=2, k=4)
            brt = smt[:][:, 76:92].rearrange("p (c e) -> p c e", c=2)
            bt = brow_t[:]
            ln1g, ln1b = bt[:, 0:4 * D], bt[:, 4 * D:8 * D]
            ln2g_o, ln2b_o = bt[:, 8 * D:10 * D], bt[:, 10 * D:12 * D]
            bv_bc = bt[:, 12 * D:13 * D]
            mrow_t = bt[:, 13 * D:13 * D + 8]

            # MoE expert-unit weight preloads: issued now so the (big) DMAs
            # overlap attention compute; tiles live in the top-level pool.
            units = []
            units.append(dict(g="A", cap=CAP_A, F=FA, tg="ua",
                              w1=t["w1_a"], w2=t["w2_a"], b1=t["b1_a"], b2=t["b2_a"],
                              yout=t["ya_t"], sel="a"))
            units.append(dict(g="B", cap=CAP_B, F=FA, tg="ub",
                              w1=t["w1_b"], w2=t["w2_b"], b1=t["b1_b"], b2=t["b2_b"],
                              yout=t["yb_t"], sel="b"))
            for u in range(3):
                units.append(dict(g="C", cap=CAP_C, F=FC, tg=f"uc{u}",
                                  w1=t["w1_c"][u], w2=t["w2_c"][u],
                                  b1=t["b1_c"][u], b2=t["b2_c"][u],
                                  yout=t["yc_t"][u], sel=f"c{u}"))
            def preload_unit(un, pool, gate):
                F, fch, tg = un["F"], un["F"] // P, un["tg"]
                un["w1_sb"] = pool.tile([P, 4, F], BF16, tag=f"w1_{tg}", name=f"w1_{tg}")
                un["w2_sb"] = pool.tile([P, fch, D], BF16, tag=f"w2_{tg}", name=f"w2_{tg}")
                # gate the big weight DMAs behind the LN1 stats so their
                # transfers can't steal bandwidth from the attention-phase
                # critical input loads (WAW dep: bypass-write, then DMA)
                for tl in (un["w1_sb"], un["w2_sb"]):
                    nc.vector.tensor_scalar(out=tl[:, 0, 0:1], in0=gate,
                                            scalar1=0.0, scalar2=None,
                                            op0=ALU.bypass)
                nc.sync.dma_start(out=un["w1_sb"][:], in_=un["w1"][:, :].rearrange("p (o f) -> p o f", o=4))
                nc.sync.dma_start(out=un["w2_sb"][:], in_=un["w2"][:, :].rearrange("p (o d) -> p o d", o=fch))
                un["b1_sb"] = pool.tile([P, fch], F32, tag=f"b1_{tg}", name=f"b1_{tg}")
                nc.sync.dma_start(out=un["b1_sb"][:], in_=un["b1"][:, :])
                un["b2_sb"] = pool.tile([P, 4], F32, tag=f"b2_{tg}", name=f"b2_{tg}")
                nc.sync.dma_start(out=un["b2_sb"][:], in_=un["b2"][:, :])
                un["eoh_sb"] = pool.tile([P, 8], F32, tag=f"eoh_{tg}", name=f"eoh_{tg}")
                nc.sync.dma_start(out=un["eoh_sb"][:, :GROUPS[un["g"]]["E"]],
                                  in_=t[f"eoh_{un['sel']}"][:, :])

            with contextlib.ExitStack() as lnx:
                lnp = lnx.enter_context(tc.tile_pool(name=R + "lnp", bufs=1))
                xln = lnp.tile([P, 4, D], F32, tag="xln")
                mvall = sm.tile([P, 4, 2], F32, tag="mvall")
                for i in range(4):
                    stats = sm.tile([P, nc.vector.BN_STATS_DIM], F32, tag="ln_stats")
                    nc.vector.bn_stats(out=stats[:], in_=x_raw[:, i, :])
                    nc.vector.bn_aggr(out=mvall[:, i, :], in_=stats[:])
                sq4 = sm.tile([P, 4], F32, tag="sq4")
                nc.scalar.activation(out=sq4[:], in_=mvall[:, :, 1], func=AF.Sqrt,
                                     bias=eps_t[:], scale=1.0)
                rs4 = sm.tile([P, 4], F32, tag="rs4")
                nc.vector.reciprocal(out=rs4[:], in_=sq4[:])
                for i in range(4):
                    nc.vector.tensor_scalar(out=xln[:, i, :], in0=x_raw[:, i, :],
                                            scalar1=mvall[:, i, 0:1],
                                            scalar2=rs4[:, i:i + 1],
                                            op0=ALU.subtract, op1=ALU.mult)
                    nc.vector.tensor_mul(out=xln[:, i, :], in0=xln[:, i, :],
                                         in1=ln1g[:, i * D:(i + 1) * D])
                    nc.vector.tensor_add(out=xln[:, i, :], in0=xln[:, i, :],
                                         in1=ln1b[:, i * D:(i + 1) * D])

                xlnT = ac.tile([P, 4, NTOK], F32R, tag="xlnT")
                for i in range(4):
                    for j in range(4):
                        pst = ps_t.tile([P, P], F32, tag="tps")
                        nc.tensor.transpose(pst[:], xln[:, i, j * P:(j + 1) * P], ident[:])
                        nc.any.tensor_copy(out=xlnT[:, j, i * P:(i + 1) * P], in_=pst[:])

            # --- QKV (fp32 data, fp32r matmuls) ---
            QT = ac.tile([P, 4, NQ], F32R, tag="QT")
            for m in range(4):
                pq = ps_mm.tile([P, NTOK], F32, tag="mm")
                for k in range(4):
                    nc.tensor.matmul(pq[:, :NQ], lhsT=(wq[:, k, m * P:(m + 1) * P]),
                                     rhs=(xlnT[:, k, 0:NQ]), start=(k == 0), stop=(k == 3))
                nc.vector.tensor_scalar(out=QT[:, m, :], in0=pq[:, :NQ],
                                        scalar1=0.125, scalar2=bq[:, m:m + 1],
                                        op0=ALU.mult, op1=ALU.add)
            KT = ac.tile([P, 4, NTOK], F32R, tag="KT")
            for m in range(4):
                pk = ps_mm.tile([P, NTOK], F32, tag="mm")
                for k in range(4):
                    nc.tensor.matmul(pk[:], lhsT=(wk[:, k, m * P:(m + 1) * P]),
                                     rhs=(xlnT[:, k, :]), start=(k == 0), stop=(k == 3))
                nc.vector.tensor_scalar(out=KT[:, m, :], in0=pk[:],
                                        scalar1=bk[:, m:m + 1], scalar2=None,
                                        op0=ALU.add)
            Vh = ac.tile([P, 4, H, HD + 1], F32R, tag="Vh")
            for m in range(4):
                pv = ps_mm.tile([P, NTOK], F32, tag="mm")
                for k in range(4):
                    nc.tensor.matmul(pv[:], lhsT=(xlnT[:, k, m * P:(m + 1) * P]),
                                     rhs=(wv[:, k, :]), start=(k == 0), stop=(k == 3))
                nc.vector.tensor_tensor(
                    out=Vh[:, m, :, 0:HD],
                    in0=pv[:].rearrange("p (h e) -> p h e", h=H),
                    in1=bv_bc.rearrange("p (h e) -> p h e", h=H),
                    op=ALU.add)
            nc.vector.tensor_copy(
                out=Vh[:, :, :, HD:HD + 1],
                in_=ones128[:, 0:32].rearrange("p (a b o) -> p a b o", a=4, b=8))

            # --- per-head attention, software-pipelined in head pairs:
            # scores of the second head are emitted before attnV of the first,
            # so the (in-order) PE queue has work while the first head's
            # softmax Exp runs on the scalar engine.
            OT = ac.tile([HD, H, NQ], F32R, tag="OT")
            for hp in range(H // 2):
                expSs = {}
                for h in (2 * hp, 2 * hp + 1):
                    hb, hc = (h % 2) * HD, h // 2
                    expS = ew.tile([P, 4, NQ], F32R, tag=f"expS{h % 2}")
                    for kc in range(4):
                        pss = ps_mm.tile([P, NTOK], F32, tag="mm")
                        nc.tensor.matmul(pss[:, :NQ],
                                         lhsT=(KT[hb:hb + HD, hc, kc * P:(kc + 1) * P]),
                                         rhs=(QT[hb:hb + HD, hc, :]),
                                         start=True, stop=True)
                        for qc in range(2):
                            nc.scalar.activation(
                                out=expS[:, kc, qc * P:(qc + 1) * P],
                                in_=pss[:, qc * P:(qc + 1) * P], func=AF.Exp,
                                bias=mrow_t[:, kc * 2 + qc:kc * 2 + qc + 1], scale=1.0)
                    expSs[h] = expS
                for h in (2 * hp, 2 * hp + 1):
                    expS = expSs[h]
                    po = ps_o.tile([HD + 1, NQ], F32, tag=f"po{h % 2}")
                    for kc in range(4):
                        nc.tensor.matmul(po[:], lhsT=(Vh[:, kc, h, :]), rhs=(expS[:, kc, :]),
                                         start=(kc == 0), stop=(kc == 3))
                    rden = sm.tile([1, NQ], F32, tag="rden")
                    nc.vector.reciprocal(out=rden[:], in_=po[HD:HD + 1, :])
                    rep = sm.tile([HD, NQ], F32, tag="rep")
                    nc.gpsimd.partition_broadcast(rep[:], rden[:], channels=HD)
                    nc.vector.tensor_mul(out=OT[:, h, :], in0=po[0:HD, :], in1=rep[:])

            # --- Wo + residual ---
            attnT = aw.tile([P, 4, NQ], F32, tag="attnT")
            for m in range(4):
                pw = ps_mm.tile([P, NTOK], F32, tag="mm")
                for h in range(H):
                    nc.tensor.matmul(pw[:, :NQ], lhsT=(woT[:, h, m * P:(m + 1) * P]),
                                     rhs=(OT[:, h, :]), start=(h == 0), stop=(h == H - 1))
                nc.vector.tensor_scalar(out=attnT[:, m, :], in0=pw[:, :NQ],
                                        scalar1=bo[:, m:m + 1], scalar2=None,
                                        op0=ALU.add)

            t_sl = aw.tile([P, 2, D], F32, tag="t_sl")
            for qt in range(2):
                for j in range(4):
                    pst = ps_t.tile([P, P], F32, tag="tps")
                    nc.tensor.transpose(pst[:], attnT[:, j, qt * P:(qt + 1) * P], ident[:])
                    nc.any.tensor_copy(out=t_sl[:, qt, j * P:(j + 1) * P], in_=pst[:])
                nc.vector.tensor_add(out=t_sl[:, qt, :], in0=t_sl[:, qt, :],
                                     in1=x_raw[:, qt, :])
            nc.sync.dma_start(out=t["t_out"][:, :].rearrange("(c p) d -> p c d", p=P),
                              in_=t_sl[:])
            # --- pre-AG: LN2 + router logits for own 2 tiles ---
            for qt in range(2):
                xg2 = aw.tile([P, D], F32, tag="xg2")
                nc.vector.tensor_copy(out=xg2[:], in_=t_sl[:, qt, :])
                _ln_tile(nc, sm, xg2[:], ln2g_o[:, qt * D:(qt + 1) * D],
                         ln2b_o[:, qt * D:(qt + 1) * D], eps_t)
                xgT2 = aw.tile([P, 4, P], F32, tag="xgT2")
                for j in range(4):
                    pst = ps_t.tile([P, P], F32, tag="tps")
                    nc.tensor.transpose(pst[:], xg2[:, j * P:(j + 1) * P], ident[:])
                    nc.any.tensor_copy(out=xgT2[:, j, :], in_=pst[:])
                pl2 = ps_x.tile([P, 8], F32, tag="plog")
                for k in range(4):
                    nc.tensor.matmul(pl2[:], lhsT=xgT2[:, k, :], rhs=wrt[:, qt, k, :],
                                     start=(k == 0), stop=(k == 3))
                lsb = aw.tile([P, 8], F32, tag="lsb")
                nc.vector.tensor_add(out=lsb[:], in0=pl2[:], in1=brt[:, qt, :])
                nc.sync.dma_start(out=agl_in[qt * P:(qt + 1) * P, :], in_=lsb[:])
                xgb = aw.tile([P, D], BF16, tag="xgb")
                nc.vector.tensor_copy(out=xgb[:], in_=xg2[:])
                nc.sync.dma_start(out=agx_in[qt * P:(qt + 1) * P, :], in_=xgb[:])

        if stage < 2:
            return
        if simulate:
            for c in range(NCORES):
                nc.sync.dma_start(out=ag_full[c * NR:(c + 1) * NR, :], in_=ag_in[:, :])
        else:
            nc.gpsimd.collective_compute(
                "AllGather", ALU.bypass,
                replica_groups=[list(range(NCORES))],
                ins=[ag_in[:, :]], outs=[ag_full[:, :]],
            )
        if stage < 3:
            return

        # ================= MoE phase =================
        with contextlib.ExitStack() as mph:
            mc = mph.enter_context(tc.tile_pool(name=R + "mc", bufs=1))
            mw = mph.enter_context(tc.tile_pool(name=R + "mw", bufs=2))
            mr = mph.enter_context(tc.tile_pool(name=R + "mr", bufs=6))

            # unit weights: issued after the collective triggers, so the
            # transfers overlap the AllGathers and don't contend with the
            # attention-phase input DMAs
            for un in units:
                preload_unit(un, wt if un["g"] != "C" else mc, rs4[:, 3:4])

            # --- routing from gathered logits, batched over all 16 tiles ---
            # padded logit columns (E..8) arrive as -1e9 (host pads brt), so
            # every op below can run on the full [P, 16, 8] block at once.
            lall = mc.tile([P, 16, 8], F32, tag="lall")
            for qv in range(2):
                nc.sync.dma_start(
                    out=lall[:].rearrange("p (c q) e -> p c q e", c=8, q=2)[:, :, qv, :],
                    in_=log_full[:, qv * P:(qv + 1) * P, :].rearrange("c p e -> p c e"))

            def bcl(ap, n):
                return bass.AP(tensor=ap.tensor, offset=ap.offset,
                               ap=list(ap.ap) + [[0, n]])

            def bcm(ap, dims):
                a = list(ap.ap)
                return bass.AP(tensor=ap.tensor, offset=ap.offset,
                               ap=a[:-1] + [[0, n] for n in dims] + [a[-1]])

            def v4(ap):
                return ap.rearrange("p (b c) e -> p b c e", c=4)

            m1a = mc.tile([P, 16], F32, tag="m1a")
            nc.vector.reduce_max(out=m1a[:], in_=lall[:], axis=mybir.AxisListType.X)
            masks1 = mc.tile([P, 16, 8], F32, tag="masks1")
            nc.vector.tensor_tensor(out=masks1[:], in0=lall[:], in1=bcl(m1a[:], 8),
                                    op=ALU.is_equal)
            tmp16 = mr.tile([P, 16, 8], F32, tag="tmp16")
            nc.vector.tensor_scalar(out=tmp16[:], in0=masks1[:], scalar1=-1e9,
                                    op0=ALU.mult, scalar2=None)
            nc.vector.tensor_add(out=tmp16[:], in0=tmp16[:], in1=lall[:])
            m2a = mc.tile([P, 16], F32, tag="m2a")
            nc.vector.reduce_max(out=m2a[:], in_=tmp16[:], axis=mybir.AxisListType.X)
            masks2 = mc.tile([P, 16, 8], F32, tag="masks2")
            nc.vector.tensor_tensor(out=masks2[:], in0=tmp16[:], in1=bcl(m2a[:], 8),
                                    op=ALU.is_equal)
            # maskor: top1 for C tiles (chunks 1, 2), top1+top2 for A/B
            mor16 = mc.tile([P, 16, 8], F32, tag="mor16")
            nc.vector.tensor_copy(out=mor16[:], in_=masks1[:])
            for ci in (0, 3):
                nc.vector.tensor_add(out=v4(mor16[:])[:, :, ci, :],
                                     in0=v4(mor16[:])[:, :, ci, :],
                                     in1=v4(masks2[:])[:, :, ci, :])

            # gates: A/B renormalized top-2 via sigmoid; C is the top-1 mask
            d12 = mr.tile([P, 16], F32, tag="d12")
            nc.vector.tensor_tensor(out=d12[:], in0=m1a[:], in1=m2a[:], op=ALU.subtract)
            g1v = mr.tile([P, 16], F32, tag="g1v")
            nc.scalar.activation(out=g1v[:], in_=d12[:], func=AF.Sigmoid)
            g2v = mr.tile([P, 16], F32, tag="g2v")
            nc.vector.tensor_scalar(out=g2v[:], in0=g1v[:], scalar1=-1.0,
                                    scalar2=1.0, op0=ALU.mult, op1=ALU.add)
            gAB = mr.tile([P, 16, 8], F32, tag="gAB")
            nc.vector.tensor_tensor(out=gAB[:], in0=masks1[:], in1=bcl(g1v[:], 8),
                                    op=ALU.mult)
            tmp2 = mr.tile([P, 16, 8], F32, tag="tmp2")
            nc.vector.tensor_tensor(out=tmp2[:], in0=masks2[:], in1=bcl(g2v[:], 8),
                                    op=ALU.mult)
            nc.vector.tensor_add(out=gAB[:], in0=gAB[:], in1=tmp2[:])
            nc.sync.dma_start(out=t["gates_a"][:, :].rearrange("(b p) e -> p b e", p=P),
                              in_=v4(gAB[:])[:, :, 0, :EA])
            nc.sync.dma_start(out=t["gates_b"][:, :].rearrange("(b p) e -> p b e", p=P),
                              in_=v4(gAB[:])[:, :, 3, :EB])
            for ci in (1, 2):
                nc.sync.dma_start(
                    out=t["gates_c"][:, :].rearrange("(b c p) e -> p b c e", p=P,
                                                     c=2)[:, :, ci - 1, :],
                    in_=v4(masks1[:])[:, :, ci, :EC])

            # --- positions: within-tile prefix (LT) + per-group tile-cumulative ---
            morf = mor16[:].rearrange("p a e -> p (a e)")
            pref = ps_mm.tile([P, NTOK], F32, tag="mm")
            nc.tensor.matmul(pref[:, :128], lhsT=LT[:], rhs=morf, start=True, stop=True)
            tots = ps_mm.tile([P, NTOK], F32, tag="mm")
            nc.tensor.matmul(tots[:, :128], lhsT=ones128[:], rhs=morf, start=True, stop=True)
            totsb = mr.tile([1, 128], F32, tag="totsb")
            nc.vector.tensor_copy(out=totsb[:], in_=tots[0:1, :128])
            cumrow = mr.tile([1, 128], F32, tag="cumrow")
            nc.vector.memset(cumrow[:], 0.0)
            for g in ("A", "C", "B"):
                gt = GROUP_TILES[g]
                for idx in range(1, len(gt)):
                    a, prv = gt[idx] * 8, gt[idx - 1] * 8
                    nc.vector.tensor_add(out=cumrow[:, a:a + 8],
                                         in0=cumrow[:, prv:prv + 8],
                                         in1=totsb[:, prv:prv + 8])
            posall16 = mc.tile([P, 16, 8], F32, tag="posall16")
            pflat = posall16[:].rearrange("p a e -> p (a e)")
            nc.vector.tensor_copy(out=pflat, in_=pref[:, :128])
            cumb = ps_mm.tile([P, NTOK], F32, tag="mm")
            nc.tensor.matmul(cumb[:, :128], lhsT=ones128[0:1, :], rhs=cumrow[:],
                             start=True, stop=True)
            nc.vector.tensor_add(out=pflat, in0=pflat, in1=cumb[:, :128])

            if stage < 4:
                return
            # --- expert units (matmul-based compaction) ---
            # (A/B weights were preloaded during attention; xbf load is
            # issued only now so the sync queue isn't blocked on the token
            # AG while routing runs)
            xbf = mc.tile([P, 16, D], BF16, tag="xbf")
            for qv in range(2):
                nc.sync.dma_start(
                    out=xbf[:].rearrange("p (c q) d -> p c q d", c=8, q=2)[:, :, qv, :],
                    in_=xbf_full[:, qv * P:(qv + 1) * P, :].rearrange("c p d -> p c d"))

            # the unit's expert id arrives as data: host encodes it by
            # pre-multiplying a one-hot (E,) selection into... simpler: the
            # expert id only affects WHICH pos/mask column is used. That
            # must be uniform across cores -> pass per-unit expert one-hot
            # as an input row and select the column via a tiny matmul.
            for un in units:
                g, cap, F, tg = un["g"], un["cap"], un["F"], un["tg"]
                E = GROUPS[g]["E"]
                gtiles = GROUP_TILES[g]
                ntiles = len(gtiles)
                fch = F // P
                w1, w2, b1, b2, eoh = (un["w1_sb"], un["w2_sb"], un["b1_sb"],
                                       un["b2_sb"], un["eoh_sb"])
                cls = "ab" if F == FA else "c"


                # G tiles: G[t, j] = (pos[t, e] == j) * maskor[t, e]
                # batched per-unit expert-column selection via one-hot dot
                pos_u = mr.tile([P, ntiles], F32, tag=f"pos_u_{cls}", name="pos_u")
                msk_u = mr.tile([P, ntiles], F32, tag=f"msk_u_{cls}", name="msk_u")
                if g == "C":
                    pv = v4(posall16[:])[:, :, 1:3, :]
                    mv = v4(mor16[:])[:, :, 1:3, :]
                    eb = bcm(eoh[:, :], (4, 2))
                    po_ = pos_u[:].rearrange("p (b c) -> p b c", c=2)
                    mo_ = msk_u[:].rearrange("p (b c) -> p b c", c=2)
                else:
                    ci = 0 if g == "A" else 3
                    pv = v4(posall16[:])[:, :, ci, :]
                    mv = v4(mor16[:])[:, :, ci, :]
                    eb = bcm(eoh[:, :], (4,))
                    po_, mo_ = pos_u[:], msk_u[:]
                tsel = mr.tile([P, ntiles, 8], F32, tag=f"tsel_{cls}", name="tsel")
                tv = (tsel[:].rearrange("p (b c) e -> p b c e", c=2)
                      if g == "C" else tsel[:])
                nc.vector.tensor_tensor(out=tv, in0=pv, in1=eb, op=ALU.mult)
                nc.vector.reduce_sum(out=po_, in_=tv, axis=mybir.AxisListType.X)
                nc.vector.tensor_tensor(out=tv, in0=mv, in1=eb, op=ALU.mult)
                nc.vector.reduce_sum(out=mo_, in_=tv, axis=mybir.AxisListType.X)
                Gts = []
                for ti in range(ntiles):
                    Gt = mw.tile([P, cap], BF16, tag=f"Gt{ti}_{cls}", name=f"Gt{ti}_{cls}")
                    nc.vector.tensor_scalar(out=Gt[:], in0=iotaCap[:, :cap],
                                            scalar1=pos_u[:, ti:ti + 1],
                                            op0=ALU.is_equal, scalar2=None)
                    nc.vector.tensor_scalar(out=Gt[:], in0=Gt[:],
                                            scalar1=msk_u[:, ti:ti + 1],
                                            op0=ALU.mult, scalar2=None)
                    Gts.append(Gt)

                # compact^T (feature-major, bf16): [128d x cap] per d-chunk
                xT = mw.tile([P, 4, cap], BF16, tag=f"xTg_{cls}", name=f"xTg_{cls}")
                for m in range(4):
                    pc = ps_mm.tile([P, NTOK], F32, tag="mm")
                    for ti in range(ntiles):
                        nc.tensor.matmul(pc[:, :cap],
                                         lhsT=xbf[:, gtiles[ti], m * P:(m + 1) * P],
                                         rhs=Gts[ti][:],
                                         start=(ti == 0), stop=(ti == ntiles - 1))
                    nc.any.tensor_copy(out=xT[:, m, :], in_=pc[:, :cap])
                if stage == 5:
                    red = sm.tile([P, 1], F32, tag="dbg_red", name="dbg_red")
                    nc.vector.reduce_sum(out=red[:], in_=xT[:], axis=mybir.AxisListType.XY)
                    nc.sync.dma_start(out=un["yout"][0:P, 0:1], in_=red[:])
                    continue

                # FFN
                hT = mw.tile([P, fch, cap], BF16, tag=f"hT_{cls}", name=f"hT_{cls}")
                for m in range(fch):
                    ph = ps_mm.tile([P, NTOK], F32, tag="mm")
                    for k in range(4):
                        nc.tensor.matmul(ph[:, :cap], lhsT=w1[:, k, m * P:(m + 1) * P],
                                         rhs=xT[:, k, :], start=(k == 0), stop=(k == 3))
                    nc.scalar.activation(out=hT[:, m, :], in_=ph[:, :cap],
                                         func=AF.Gelu_apprx_tanh,
                                         bias=b1[:, m:m + 1], scale=1.0)
                if stage == 6:
                    red = sm.tile([P, 1], F32, tag="dbg_red", name="dbg_red")
                    nc.vector.reduce_sum(out=red[:], in_=hT[:], axis=mybir.AxisListType.XY)
                    nc.sync.dma_start(out=un["yout"][0:P, 0:1], in_=red[:])
                    continue
                y = mw.tile([P, 4, cap], BF16, tag=f"y_{cls}", name=f"y_{cls}")
                for m in range(4):
                    py = ps_mm.tile([P, NTOK], F32, tag="mm")
                    for k in range(fch):
                        nc.tensor.matmul(py[:, :cap], lhsT=w2[:, k, m * P:(m + 1) * P],
                                         rhs=hT[:, k, :], start=(k == 0), stop=(k == fch - 1))
                    nc.scalar.activation(out=y[:, m, :], in_=py[:, :cap], func=AF.Identity,
                                         bias=b2[:, m:m + 1], scale=1.0)
                nc.gpsimd.dma_start(out=un["yout"][:, :].rearrange("p (o c) -> p o c", o=4),
                                  in_=y[:])


@functools.lru_cache(maxsize=8)
def _build(stage=4, reps=1, simulate=False):
    nc = bacc.Bacc("TRN2", target_bir_lowering=False, debug=False,
                   num_devices=1 if simulate else NCORES,
                   enable_asserts=False)
    t = _declare_io(nc)
    with tile.TileContext(nc) as tc:
        for r in range(reps):
            _body(nc, tc, t, stage=stage, rep=r, simulate=simulate)
    nc.compile()
    return nc


# --------------------------------------------------------------------------
# host side
# --------------------------------------------------------------------------

def _seg_mask():
    seg = np.concatenate([np.zeros(128), np.ones(256), 2 * np.ones(128)])
    allowed = seg[None, :] <= seg[:, None]  # (q, k)
    return np.where(allowed, 0.0, NEG_MASK).astype(np.float32)


def _host_inputs(inputs):
    """Build the 8 per-core input maps."""
    f32 = np.float32
    bf = ml_dtypes.bfloat16
    tokens_A = np.asarray(inputs["tokens_A"], f32)
    tokens_B = np.asarray(inputs["tokens_B"], f32)
    tokens_C = np.asarray(inputs["tokens_C"], f32)
    ln1_g = np.asarray(inputs["ln1_g"], f32); ln1_b = np.asarray(inputs["ln1_b"], f32)
    ln2_g = np.asarray(inputs["ln2_g"], f32); ln2_b = np.asarray(inputs["ln2_b"], f32)
    Wqkv = np.asarray(inputs["Wqkv"], f32); bqkv = np.asarray(inputs["bqkv"], f32)
    Wo = np.asarray(inputs["Wo"], f32); bo = np.asarray(inputs["bo"], f32)

    X = np.concatenate([tokens_A, tokens_C, tokens_B], axis=1)  # (B, 512, D)

    def pb(x):
        x = np.asarray(x, f32)
        return np.ascontiguousarray(np.broadcast_to(x.reshape(1, -1), (128, x.size)))

    def tile_rows(x, p=128):
        # (o*p, X) -> (p, o*X) with [p, o, X] layout (per-partition contiguous)
        x = np.asarray(x)
        o = x.shape[0] // p
        return np.ascontiguousarray(
            x.reshape(o, p, -1).transpose(1, 0, 2).reshape(p, -1))

    wr_pad = {}
    br_pad = {}
    for g, wr_k, br_k in (("A", "Wr_A", "br_A"), ("B", "Wr_B", "br_B"), ("C", "Wr_C", "br_C")):
        E = {"A": EA, "B": EB, "C": EC}[g]
        wrp = np.zeros((D, 8), f32); wrp[:, :E] = np.asarray(inputs[wr_k], f32)
        brp = np.full(8, -1e9, f32); brp[:E] = np.asarray(inputs[br_k], f32)
        wr_pad[g] = wrp; br_pad[g] = brp

    attn_w = np.concatenate([tile_rows(Wqkv.T[:, 0:D]),
                             tile_rows(Wqkv.T[:, D:2 * D]),
                             tile_rows(Wqkv.T[:, 2 * D:3 * D])], axis=1)
    bq_r = np.ascontiguousarray((bqkv[:512] * 0.125).reshape(4, 128).T)
    bk_r = np.ascontiguousarray(bqkv[512:1024].reshape(4, 128).T)
    bo_r = np.ascontiguousarray(bo.reshape(4, 128).T)
    bv_row = bqkv[1024:].reshape(1, D)
    base = dict(attn_w=attn_w, woT=tile_rows(Wo.T, p=64))
    W1_A = np.asarray(inputs["W1_A"], f32); W2_A = np.asarray(inputs["W2_A"], f32)
    b1_A = np.asarray(inputs["b1_A"], f32); b2_A = np.asarray(inputs["b2_A"], f32)
    W1_B = np.asarray(inputs["W1_B"], f32); W2_B = np.asarray(inputs["W2_B"], f32)
    b1_B = np.asarray(inputs["b1_B"], f32); b2_B = np.asarray(inputs["b2_B"], f32)
    W1_C = np.asarray(inputs["W1_C"], f32); W2_C = np.asarray(inputs["W2_C"], f32)
    b1_C = np.asarray(inputs["b1_C"], f32); b2_C = np.asarray(inputs["b2_C"], f32)

    chunk_groups = [["A", "C", "C", "B"], ["C", "B", "A", "C"]]
    gidx = {"A": 0, "C": 1, "B": 2}

    seg_of = {"A": 0, "C": 1, "B": 2}
    in_maps = []
    for c in range(NCORES):
        b, s = c // 2, c % 2
        xb = np.roll(X[b], -s * 256, axis=0)  # queries at rows 0:256
        # chunk-level mask table in rolled coords: mrow[kc*2+qc]
        segs = [seg_of[g] for g in chunk_groups[s]]
        mrow = np.zeros((1, 8), f32)
        for kc in range(4):
            for qc in range(2):
                mrow[0, kc * 2 + qc] = 0.0 if segs[kc] <= segs[qc] else NEG_MASK
        g1c = np.stack([ln1_g[gidx[g]] for g in chunk_groups[s]]).reshape(1, 4 * D)
        b1c = np.stack([ln1_b[gidx[g]] for g in chunk_groups[s]]).reshape(1, 4 * D)
        own_groups = chunk_groups[s][:2]
        g2c = np.stack([ln2_g[gidx[g]] for g in own_groups]).reshape(1, 2 * D)
        b2c = np.stack([ln2_b[gidx[g]] for g in own_groups]).reshape(1, 2 * D)
        brow = np.concatenate([g1c, b1c, g2c, b2c, bv_row.astype(f32), mrow],
                              axis=1)
        # per-own-tile padded router weights: (P, 2*4*8) / (P, 2*8)
        wrt = np.stack([tile_rows(wr_pad[g]).reshape(128, 4, 8) for g in own_groups],
                       axis=1).reshape(128, 2 * 4 * 8)
        brt = np.stack([np.broadcast_to(br_pad[g], (128, 8)) for g in own_groups],
                       axis=1).reshape(128, 2 * 8)

        eA = c // 2; fA = c % 2
        eB = c // 2; fB = c % 2
        cu = [(3 * c + u) // 4 for u in range(3)]   # C experts per unit
        cq = [(3 * c + u) % 4 for u in range(3)]    # C quarter per unit
        smalls = np.concatenate([bq_r, bk_r, bo_r, wrt.reshape(128, 64),
                                 brt.reshape(128, 16)], axis=1)
        m = dict(base)
        m.update(
            x_tok=np.ascontiguousarray(xb),
            brow=brow, smalls=smalls,
            w1_a=tile_rows(W1_A[eA][:, fA * FA:(fA + 1) * FA]).astype(bf),
            w2_a=tile_rows(W2_A[eA][fA * FA:(fA + 1) * FA, :]).astype(bf),
            b1_a=np.ascontiguousarray(b1_A[eA][fA * FA:(fA + 1) * FA].reshape(FA // P, P).T),
            b2_a=np.ascontiguousarray((b2_A[eA] if fA == 0 else np.zeros(D, f32)).reshape(4, P).T),
            w1_b=tile_rows(W1_B[eB][:, fB * FA:(fB + 1) * FA]).astype(bf),
            w2_b=tile_rows(W2_B[eB][fB * FA:(fB + 1) * FA, :]).astype(bf),
            b1_b=np.ascontiguousarray(b1_B[eB][fB * FA:(fB + 1) * FA].reshape(FA // P, P).T),
            b2_b=np.ascontiguousarray((b2_B[eB] if fB == 0 else np.zeros(D, f32)).reshape(4, P).T),
            w1_c=np.stack([tile_rows(W1_C[cu[u]][:, cq[u] * FC:(cq[u] + 1) * FC]) for u in range(3)]).astype(bf),
            w2_c=np.stack([tile_rows(W2_C[cu[u]][cq[u] * FC:(cq[u] + 1) * FC, :]) for u in range(3)]).astype(bf),
            b1_c=np.stack([b1_C[cu[u]][cq[u] * FC:(cq[u] + 1) * FC].reshape(FC // P, P).T for u in range(3)]),
            b2_c=np.stack([(b2_C[cu[u]] if cq[u] == 0 else np.zeros(D, f32)).reshape(4, P).T for u in range(3)]),
            eoh_a=pb(np.eye(EA, dtype=f32)[eA]),
            eoh_b=pb(np.eye(EB, dtype=f32)[eB]),
            eoh_c0=pb(np.eye(EC, dtype=f32)[cu[0]]),
            eoh_c1=pb(np.eye(EC, dtype=f32)[cu[1]]),
            eoh_c2=pb(np.eye(EC, dtype=f32)[cu[2]]),
        )
        in_maps.append({k: np.ascontiguousarray(v) for k, v in m.items()})
    return in_maps


def _combine(results):
    """Host combine: out = t_full + sum gate * y, slots derived from gates."""
    f32 = np.float32
    t_full = np.concatenate([results[c]["t_out"] for c in range(NCORES)], axis=0)
    out = t_full.astype(f32).copy()

    def apply_unit(ytab, gates, e, cap, goff, gwidth):
        sel = gates[:, e] > 0
        gr = np.nonzero(sel)[0][:cap]
        if gr.size == 0:
            return
        rows = (gr // gwidth) * NTOK + goff + (gr % gwidth)
        gate = gates[gr, e]
        # ytab (128, 4*cap) with [p, o, cap] layout -> y^T (512, cap)
        yt = ytab.reshape(P, 4, cap).transpose(1, 0, 2).reshape(4 * P, cap)
        y = yt.T[:gr.size].astype(f32)
        np.add.at(out, rows, gate[:, None].astype(f32) * y)

    for c in range(NCORES):
        r = results[c]
        apply_unit(r["ya_t"], r["gates_a"], c // 2, CAP_A, 0, 128)
        apply_unit(r["yb_t"], r["gates_b"], c // 2, CAP_B, 384, 128)
        for u in range(3):
            cu = (3 * c + u) // 4
            apply_unit(r["yc_t"][u], r["gates_c"], cu, CAP_C, 128, 256)

    return out.reshape(BATCH, NTOK, D)


_LAST_RESULTS = None
_EXEC_CACHE = {}


def _get_exec(stage=4, reps=1):
    """Build (once) a cached jitted shard_map executable for the NEFF."""
    if (stage, reps) in _EXEC_CACHE:
        return _EXEC_CACHE[(stage, reps)]
    import jax
    from jax.sharding import Mesh, PartitionSpec
    from jax.experimental.shard_map import shard_map
    from concourse.bass2jax import install_neuronx_cc_hook, _bass_exec_p
    import concourse.mybir as _mybir

    nc = _build(stage, reps)
    install_neuronx_cc_hook()
    in_names, out_names, out_avals, zero_outs = [], [], [], []
    assert nc.dbg_addr is None
    partition_name = nc.partition_id_tensor.name if nc.partition_id_tensor else None
    for alloc in nc.m.functions[0].allocations:
        if not isinstance(alloc, _mybir.MemoryLocationSet):
            continue
        name = alloc.memorylocations[0].name
        if alloc.kind == "ExternalInput":
            if name != partition_name:
                in_names.append(name)
        elif alloc.kind == "ExternalOutput":
            shape = tuple(alloc.tensor_shape)
            dtype = _mybir.dt.np(alloc.dtype)
            out_names.append(name)
            out_avals.append(jax.core.ShapedArray(shape, dtype))
            zero_outs.append(np.zeros(shape, dtype))
    n_params = len(in_names)
    all_names = in_names + out_names
    if partition_name is not None:
        all_names = all_names + [partition_name]

    def _fn(*args):
        from concourse.bass2jax import partition_id_tensor as _pid
        operands = list(args)
        if partition_name is not None:
            operands.append(_pid())
        outs = _bass_exec_p.bind(
            *operands,
            out_avals=tuple(out_avals),
            in_names=tuple(all_names),
            out_names=tuple(out_names),
            lowering_input_output_aliases=(),
            sim_require_finite=True,
            sim_require_nnan=True,
            nc=nc,
        )
        return tuple(outs)

    devices = jax.devices()[:NCORES]
    mesh = Mesh(np.asarray(devices), ("core",))
    nin = n_params + len(out_names)
    sharded = jax.jit(
        shard_map(_fn, mesh=mesh, in_specs=(PartitionSpec("core"),) * nin,
                  out_specs=(PartitionSpec("core"),) * len(out_names),
                  check_rep=False),
        keep_unused=True)
    _EXEC_CACHE[(stage, reps)] = dict(
        fn=sharded, in_names=in_names, out_names=out_names,
        out_avals=out_avals, zero_outs=zero_outs, mesh=mesh, stage=(stage, reps))
    return _EXEC_CACHE[(stage, reps)]


def _concat_inputs(in_maps, ex):
    concat = [np.concatenate([np.asarray(in_maps[c][nm]) for c in range(NCORES)], axis=0)
              for nm in ex["in_names"]]
    zeros = [np.zeros((NCORES * z.shape[0], *z.shape[1:]), z.dtype)
             for z in ex["zero_outs"]]
    return concat + zeros


def _run(in_maps):
    ex = _get_exec()
    args = _concat_inputs(in_maps, ex)
    out_arrs = ex["fn"](*args)
    results = []
    for c in range(NCORES):
        r = {}
        for i, nm in enumerate(ex["out_names"]):
            shp = ex["out_avals"][i].shape
            r[nm] = np.asarray(out_arrs[i]).reshape(NCORES, *shp)[c]
        results.append(r)
    return results


def kernel(**inputs):
    global _LAST_RESULTS
    in_maps = _host_inputs(inputs)
    results = _run(in_maps)
    _LAST_RESULTS = results
    return _combine(results)

